# revision 17
# baseline (speedup 1.0000x reference)
"""Trainium2 Bass kernel for nn_MultiHeadAttention_68152541053005.

Multi-head attention (B=2, N=2048, D=1024, H=16, d=64) with RoPE,
per-head RMSNorm on q/k, per-dim scale on q, causal softmax.

Sharding: 8 cores = 2 batch groups x 4 head-groups (4 heads/core).
Each core computes QKV projection for its 4 heads on its batch,
attention, and a partial output projection; the host sums the 4
partial outputs per batch (equivalent to the all-reduce after the
output projection).

Per-core kernel, software-pipelined across engines:
  - single interleaved instruction stream: QKV projection for token
    group g+1 is woven between attention matmuls for q-block g, so the
    PE never idles while ACT runs softmax exp (and stays at its max
    p-state, which needs ~3us of continuous PE work)
  - qT/kT produced by DMA XBAR transposes (no PE transposes)
  - ACT does exp + qk psum eviction only; DVE does psum-touching ops
    (v evict, reduce, rsqrt, recip, normalize, outproj evict); the
    Pool engine does sbuf-only elementwise (RoPE, squares, per-dim
    scale, causal masks)
  - softmax denominator rides as a ones-row in the ctx matmul; its
    reciprocal is partition-broadcast via a K=1 PE matmul
  - PSUM plan (8 banks): pqkv[128,768]f32 x2 bufs (4) + st[128,1024]
    f32 x1 (2) + ctx[128,512] x1 (1) + misc(po/bcast)[128,512] x1 (1)
"""

import os
import sys

if "/opt/trn_rl_repo" not in sys.path:
    sys.path.insert(0, "/opt/trn_rl_repo")

import numpy as np
from contextlib import ExitStack

import concourse.bacc as bacc
import concourse.bass as bass
import concourse.mybir as mybir
import concourse.tile as tile

AP = bass.AP
F32 = mybir.dt.float32
BF16 = mybir.dt.bfloat16
AFT = mybir.ActivationFunctionType
ALU = mybir.AluOpType

B, N, D, H, HD = 2, 2048, 1024, 16, 64
NH = 4            # heads per core
HALF = HD // 2    # 32
TC = N // 128     # 16 token chunks
DC = D // 128     # 8 D chunks
QB = N // 512     # 4 q blocks
LOG2_E = 1.442695041
RMS_EPS = 1e-6
MAX_TIMESCALE = 10000.0

# rsqrt(v) on DVE: z0 = c2*(v+h)^2 + k, then 2 Newton steps
# z <- z*(1.5 - 0.5*v*z^2); max rel err 8.5e-5 on v in [0.3, 2.3]
RS_H = -2.0157414099271302
RS_K = 0.6774616747941173
RS_C2 = 0.34740916
RS_VLO, RS_VHI = 0.3, 2.3


def _np_bf16():
    import ml_dtypes
    return np.dtype(ml_dtypes.bfloat16)


def build_nc():
    nc = bacc.Bacc("TRN2", target_bir_lowering=False, debug=False)

    x_d = nc.dram_tensor("x", [D, N], BF16, kind="ExternalInput")
    wqkv_d = nc.dram_tensor("wqkv", [D, 3 * NH * HD], BF16, kind="ExternalInput")
    wo_d = nc.dram_tensor("wo", [NH * HD, D], BF16, kind="ExternalInput")
    ctab_d = nc.dram_tensor("ctab", [N, 8 * HALF], BF16, kind="ExternalInput")
    masks_d = nc.dram_tensor("masks", [128, 128], BF16, kind="ExternalInput")
    ident_d = nc.dram_tensor("ident", [128, 128], BF16, kind="ExternalInput")
    outT_d = nc.dram_tensor("outT", [D, N], F32, kind="ExternalOutput")

    with tile.TileContext(nc) as tc, ExitStack() as ctx:
        build_tile_kernel(ctx, tc, x_d.ap(), wqkv_d.ap(), wo_d.ap(),
                          ctab_d.ap(), masks_d.ap(), ident_d.ap(),
                          outT_d.ap())
    nc.compile()
    return nc


def build_tile_kernel(ctx, tc, x, wqkv, wo, ctab, masks, identD, outT):
    nc = tc.nc

    res = ctx.enter_context(tc.tile_pool(name="res", bufs=1))
    stream = ctx.enter_context(tc.tile_pool(name="stream", bufs=3))
    scratch = ctx.enter_context(tc.tile_pool(name="scratch", bufs=2))
    qkpool = ctx.enter_context(tc.tile_pool(name="qkpool", bufs=3))
    ptp = ctx.enter_context(tc.tile_pool(name="ptp", bufs=3))
    ropep = ctx.enter_context(tc.tile_pool(name="ropep", bufs=5))

    pq = ctx.enter_context(tc.tile_pool(name="pq", bufs=1, space="PSUM"))
    pS = ctx.enter_context(tc.tile_pool(name="pS", bufs=2, space="PSUM"))
    pC = ctx.enter_context(tc.tile_pool(name="pC", bufs=1, space="PSUM"))
    pM = ctx.enter_context(tc.tile_pool(name="pM", bufs=1, space="PSUM"))

    # ---- resident constants ----
    wqkv_sb = res.tile([128, DC * 768], BF16, tag="wqkv")
    for c in range(DC):
        nc.sync.dma_start(wqkv_sb[:, 768 * c:768 * (c + 1)],
                          wqkv[128 * c:128 * (c + 1), :])
    wo_sb = res.tile([128, 2 * D], BF16, tag="wo")
    for r in range(2):
        nc.sync.dma_start(wo_sb[:, D * r:D * (r + 1)],
                          wo[128 * r:128 * (r + 1), :])
    masks_sb = res.tile([128, 128], BF16, tag="masks")
    nc.sync.dma_start(masks_sb[:], masks[:])
    ident = res.tile([128, 128], BF16, tag="ident")
    nc.sync.dma_start(ident[:], identD[:])

    qkT_all = res.tile([128, 4 * N], BF16, tag="qkT_all")
    qT = [qkT_all[:, i * N:(i + 1) * N] for i in range(2)]
    kT = [qkT_all[:, (2 + i) * N:(3 + i) * N] for i in range(2)]
    ctxT = [res.tile([128, N], BF16, tag=f"ctxT{i}", name=f"ctxT{i}")
            for i in range(2)]
    vt = res.tile([128, TC * NH * 65], BF16, tag="vt")
    va = vt[:]
    ones_dst = AP(va.tensor, va.offset + HD,
                  [va.ap[0], [NH * 65, TC], [65, NH], [1, 1]])
    nc.vector.memset(ones_dst, 1.0)

    # x is [D, N] in DRAM; stage the token-chunk columns in waves so
    # early chunks unblock quickly
    xT_big = [res.tile([128, N], BF16, tag=f"xT{c}", name=f"xTbig{c}")
              for c in range(DC)]
    for lo, hi in ((0, 128), (128, 512), (512, 1024), (1024, 2048)):
        for c in range(DC):
            nc.sync.dma_start(xT_big[c][:, lo:hi],
                              x[128 * c:128 * (c + 1), lo:hi])

    # ------------------------------------------------------------------
    # chunk pipeline: QKV projection + postproc for token chunk t
    # ------------------------------------------------------------------
    group_state = {}

    def make_chunk_units(t):
        state = {}
        gi, dt_i = divmod(t, 4)

        def mk(c):
            def u():
                if c == 0:
                    ctab_t = stream.tile([128, 256], BF16, tag="ctab",
                                         name=f"ctab{t}")
                    nc.sync.dma_start(ctab_t[:],
                                      ctab[128 * t:128 * (t + 1), :])
                    state["ctab"] = ctab_t
                    state["pqkv"] = pq.tile([128, 768], F32, tag="pqkv",
                                            name=f"pqkv{t}")
                pqkv = state["pqkv"]
                lhsT = xT_big[c][:, 128 * t:128 * (t + 1)]
                nc.tensor.matmul(pqkv[:, 0:512],
                                 lhsT, wqkv_sb[:, 768 * c:768 * c + 512],
                                 start=(c == 0), stop=(c == DC - 1))
                nc.tensor.matmul(pqkv[:, 512:768],
                                 lhsT, wqkv_sb[:, 768 * c + 512:768 * (c + 1)],
                                 start=(c == 0), stop=(c == DC - 1))
                if c == DC - 1:
                    post(state["pqkv"], state["ctab"])
            return (768, u)

        def post(pqkv, ctab_t):
            # qk eviction (ACT), v eviction (DVE, strided with ones gaps)
            qk_sb = qkpool.tile([128, 512], BF16, tag="qk_sb",
                                name=f"qk_sb{t}")
            nc.scalar.copy(qk_sb[:], pqkv[:, 0:512])
            va2 = vt[:]
            v_dst = AP(va2.tensor, va2.offset + NH * 65 * t,
                       [va2.ap[0], [65, NH], [1, HD]])
            pa = pqkv[:]
            v_src = AP(pa.tensor, pa.offset + 512,
                       [pa.ap[0], [HD, NH], [1, HD]])
            nc.vector.tensor_copy(v_dst, v_src)

            # RMSNorm stats from pre-RoPE q/k (rotation preserves norms);
            # reduce into the group's [128, 32] stats tile
            sq = scratch.tile([128, 512], BF16, tag="sq", name=f"sq{t}")
            nc.gpsimd.tensor_mul(sq[:], qk_sb[:], qk_sb[:])
            if dt_i == 0:
                group_state[gi] = {
                    "ssq": scratch.tile([128, 32], F32, tag="ssq",
                                        name=f"ssq_g{gi}")
                }
            ssq = group_state[gi]["ssq"]
            nc.vector.reduce_sum(ssq[:, 8 * dt_i:8 * (dt_i + 1)],
                                 sq[:].rearrange("p (h d) -> p h d", d=HD),
                                 axis=mybir.AxisListType.X)

            # RoPE on DVE via host-folded tables (per-dim scales folded in)
            def dat(off, tl):
                a = tl[:]
                return AP(a.tensor, a.offset + off,
                          [a.ap[0], [256, 2], [HD, NH], [1, HALF]])

            def tab(f):
                a = ctab_t[:]
                return AP(a.tensor, a.offset + 64 * f,
                          [a.ap[0], [HALF, 2], [0, NH], [1, HALF]])

            tmp = [scratch.tile([128, 256], BF16, tag=f"rp{i}",
                                name=f"rp{i}_{t}") for i in range(4)]
            roped = ropep.tile([128, 512], BF16, tag="roped",
                               name=f"roped{t}")
            nc.vector.tensor_mul(tmp[0][:], dat(0, qk_sb), tab(0))
            nc.vector.tensor_mul(tmp[1][:], dat(HALF, qk_sb), tab(1))
            nc.vector.tensor_sub(dat(0, roped), tmp[0][:], tmp[1][:])
            nc.vector.tensor_mul(tmp[2][:], dat(HALF, qk_sb), tab(2))
            nc.vector.tensor_mul(tmp[3][:], dat(0, qk_sb), tab(3))
            nc.vector.tensor_add(dat(HALF, roped), tmp[2][:], tmp[3][:])
            group_state[gi][f"roped{dt_i}"] = roped

            if dt_i == 3:
                # batched rsqrt for the group (quadratic seed + 2 Newton)
                v_ = scratch.tile([128, 32], F32, tag="rsv", name=f"rsv{gi}")
                nc.vector.tensor_scalar(v_[:], ssq[:], 1.0 / HD, RMS_EPS,
                                        ALU.mult, ALU.add)
                vc = scratch.tile([128, 32], F32, tag="rsvc",
                                  name=f"rsvc{gi}")
                nc.vector.tensor_scalar(vc[:], v_[:], RS_VLO, RS_VHI,
                                        ALU.max, ALU.min)
                t_ = scratch.tile([128, 32], F32, tag="rst", name=f"rst{gi}")
                nc.vector.tensor_scalar_add(t_[:], vc[:], RS_H)
                z_ = scratch.tile([128, 32], F32, tag="rsz", name=f"rsz{gi}")
                nc.vector.scalar_tensor_tensor(z_[:], t_[:], RS_C2, t_[:],
                                               ALU.mult, ALU.mult)
                nc.vector.tensor_scalar_add(z_[:], z_[:], RS_K)
                z2 = scratch.tile([128, 32], F32, tag="rsz2",
                                  name=f"rsz2{gi}")
                w_ = scratch.tile([128, 32], F32, tag="rsw", name=f"rsw{gi}")
                rs = scratch.tile([128, 32], F32, tag="rs_g", name=f"rs{gi}")
                for it in range(2):
                    nc.vector.tensor_mul(z2[:], z_[:], z_[:])
                    nc.vector.scalar_tensor_tensor(w_[:], z2[:], -0.5, v_[:],
                                                   ALU.mult, ALU.mult)
                    out_ = rs if it == 1 else z_
                    nc.vector.scalar_tensor_tensor(out_[:], w_[:], 1.5, z_[:],
                                                   ALU.add, ALU.mult)
                group_state[gi]["rs"] = rs

        return [mk(c) for c in range(DC)]

    def make_trans_unit(t):
        gi, dt_i = divmod(t, 4)

        def u():
            roped = group_state[gi].pop(f"roped{dt_i}")
            rs = group_state[gi]["rs"]
            # apply per-(token, head) rsqrt; broadcast over the 64 dims
            qk_stage = scratch.tile([128, 512], BF16, tag="qk_stage",
                                    name=f"qk_stage{t}")
            ra = rs[:]
            rs_b = AP(ra.tensor, ra.offset + 8 * dt_i,
                      [ra.ap[0], [1, 8], [0, HD]])
            nc.gpsimd.tensor_mul(
                qk_stage[:].rearrange("p (h d) -> p h d", d=HD),
                roped[:].rearrange("p (h d) -> p h d", d=HD), rs_b)
            ptq = pM.tile([128, 512], BF16, tag="misc", name=f"ptq{t}")
            for i in range(4):
                nc.tensor.transpose(ptq[:, 128 * i:128 * (i + 1)],
                                    qk_stage[:, 128 * i:128 * (i + 1)],
                                    ident[:])
            qa_ = qkT_all[:]
            dst = AP(qa_.tensor, qa_.offset + 128 * t,
                     [qa_.ap[0], [N, 4], [1, 128]])
            nc.vector.tensor_copy(dst, ptq[:])
        return (512, u)

    # ------------------------------------------------------------------
    # attention for q block Q (512 queries), head h
    # ------------------------------------------------------------------
    def make_attn_units(Q):
        qcol = slice(512 * Q, 512 * (Q + 1))
        units = []
        for h in range(NH):
            g, off = divmod(h, 2)
            row = slice(64 * off, 64 * off + 64)
            npair = 2 * Q + 2
            st_state = {}

            def mk_st(p, g=g, row=row, h=h):
                def u():
                    pst = pS.tile([128, 1024], F32, tag="st",
                                  name=f"st{Q}_{h}_{p}")
                    regions = []
                    for s in range(2):
                        j = 2 * p + s
                        qoff = max(0, 128 * j - 512 * Q)
                        cols = 512 - qoff
                        nc.tensor.matmul(
                            pst[:, 512 * s:512 * s + cols],
                            kT[g][row, 128 * j:128 * (j + 1)],
                            qT[g][row, 512 * Q + qoff:512 * (Q + 1)],
                            start=True, stop=True)
                        regions.append(cols)
                    pt = ptp.tile([128, 1024], BF16, tag="pt",
                                  name=f"pt{Q}_{h}_{p}")
                    c0, c1 = regions
                    if c0 == 512:
                        nc.scalar.activation(pt[:, 0:512 + c1],
                                             pst[:, 0:512 + c1], AFT.Exp)
                    else:
                        nc.scalar.activation(pt[:, 0:c0], pst[:, 0:c0],
                                             AFT.Exp)
                        nc.scalar.activation(pt[:, 512:512 + c1],
                                             pst[:, 512:512 + c1], AFT.Exp)
                    if p >= 2 * Q:
                        nc.vector.tensor_mul(pt[:, 0:128], pt[:, 0:128],
                                             masks_sb[:])
                        nc.vector.tensor_mul(pt[:, 512:640], pt[:, 512:640],
                                             masks_sb[:])
                    st_state[p] = pt
                cost = sum(512 - max(0, 128 * (2 * p + s) - 512 * Q)
                           for s in range(2))
                return (cost, u)

            def mk_ctx(p, h=h, npair=npair):
                def u():
                    if p == 0:
                        st_state["pctx"] = pC.tile([65, 512], F32, tag="ctx",
                                                   name=f"ctx{Q}_{h}")
                    pctx = st_state["pctx"]
                    pt = st_state.pop(p)
                    for s in range(2):
                        j = 2 * p + s
                        qoff = max(0, 128 * j - 512 * Q)
                        cols = 512 - qoff
                        nc.tensor.matmul(
                            pctx[:, qoff:512],
                            vt[:, 65 * (NH * j + h):65 * (NH * j + h) + 65],
                            pt[:, 512 * s:512 * s + cols],
                            start=(j == 0), stop=(j == 2 * npair - 1))
                cost = sum(512 - max(0, 128 * (2 * p + s) - 512 * Q)
                           for s in range(2))
                return (cost, u)

            def mk_norm(g=g, row=row, h=h):
                def u():
                    pctx = st_state.pop("pctx")
                    # evict raw ctx+den to SBUF in one op to free the psum
                    # bank fast; normalization happens off the critical path
                    u_sb = scratch.tile([65, 512], F32, tag="u_sb",
                                        name=f"u{Q}_{h}", bufs=3)
                    nc.vector.tensor_copy(u_sb[:], pctx[:])
                    den_sb = scratch.tile([1, 512], F32, tag="den_sb",
                                          name=f"den{Q}_{h}")
                    nc.vector.tensor_copy(den_sb[:], pctx[64:65, :])
                    recip0 = scratch.tile([1, 512], F32, tag="recip0",
                                          name=f"recip0_{Q}_{h}")
                    nc.vector.reciprocal_approx_fast(recip0[:], den_sb[:])
                    bcast = scratch.tile([64, 512], F32, tag="bcast",
                                         name=f"bcast{Q}_{h}")
                    nc.gpsimd.partition_broadcast(bcast[:], recip0[:])
                    nc.gpsimd.tensor_mul(ctxT[g][row, qcol],
                                         u_sb[0:64, :], bcast[:])
                return (640, u)

            units.append(mk_st(0))
            for p in range(1, npair):
                units.append(mk_st(p))
                units.append(mk_ctx(p - 1))
            units.append(mk_ctx(npair - 1))
            units.append(mk_norm())
        return units

    # ------------------------------------------------------------------
    # output projection for q block Q
    # ------------------------------------------------------------------
    def make_outproj_units(Q):
        qcol = slice(512 * Q, 512 * (Q + 1))
        units = []
        for m in range(DC):
            def u(m=m):
                po = pM.tile([128, 512], F32, tag="misc",
                             name=f"po{Q}_{m}")
                for r in range(2):
                    nc.tensor.matmul(
                        po[:],
                        wo_sb[:, D * r + 128 * m:D * r + 128 * (m + 1)],
                        ctxT[r][:, qcol], start=(r == 0), stop=(r == 1))
                ob = scratch.tile([128, 512], F32, tag="ob",
                                  name=f"ob{Q}_{m}")
                if m % 2 == 0:
                    nc.scalar.copy(ob[:], po[:])
                else:
                    nc.vector.tensor_copy(ob[:], po[:])
                nc.sync.dma_start(outT[128 * m:128 * (m + 1), qcol], ob[:])
            units.append((1024, u))
        return units

    # ------------------------------------------------------------------
    # weave two unit streams proportionally by PE cost
    # ------------------------------------------------------------------
    def weave(A, B):
        totA = sum(c for c, _ in A) or 1
        totB = sum(c for c, _ in B) or 1
        out = []
        ia = ib = 0
        ca = cb = 0
        while ia < len(A) or ib < len(B):
            fa = ca / totA
            fb = cb / totB
            if ib >= len(B) or (ia < len(A) and fa <= fb):
                c, u = A[ia]; ia += 1; ca += c
            else:
                c, u = B[ib]; ib += 1; cb += c
            out.append(u)
        return out

    def make_group_units(g):
        units = []
        for t in range(4 * g, 4 * (g + 1)):
            units += make_chunk_units(t)
        for t in range(4 * g, 4 * (g + 1)):
            units.append(make_trans_unit(t))
        return units

    plan = []
    # group 0 alone (attention needs its k/v first)
    plan += [u for _, u in make_group_units(0)]
    # attn(Q) ∥ chunks of group Q+1 ∥ outproj(Q-1)
    for Q in range(3):
        other = make_attn_units(Q) + (make_outproj_units(Q - 1) if Q else [])
        plan += weave(other, make_group_units(Q + 1))
    plan += weave(make_attn_units(3), make_outproj_units(2))
    plan += [u for _, u in make_outproj_units(3)]

    for u in plan:
        u()


# ---------------------------------------------------------------------------
# host side
# ---------------------------------------------------------------------------

_CACHE = {}


def _get_nc():
    if "nc" not in _CACHE:
        _CACHE["nc"] = build_nc()
    return _CACHE["nc"]


def _host_tables(q_ln_scale, k_ln_scale, per_dim_scale):
    frac = 2.0 * np.arange(HALF, dtype=np.float32) / HD
    ts = (MAX_TIMESCALE ** frac).astype(np.float32)
    pos = np.arange(N, dtype=np.float32)
    sinu = pos[:, None] / ts[None, :]
    SIN = np.sin(sinu).astype(np.float32)
    COS = np.cos(sinu).astype(np.float32)
    qs = (LOG2_E / np.sqrt(np.float32(HD))
          * np.logaddexp(0.0, per_dim_scale.astype(np.float64))).astype(np.float32)
    qscale = (q_ln_scale * qs).astype(np.float32)
    kscale = k_ln_scale.astype(np.float32)

    # combined table [N, 256]: func f in {cosA,sinA,cosB,sinB} at cols
    # [64f:64f+64], q-scaled half at +0:32, k-scaled at +32:64
    blocks = []
    for base, half in ((COS, slice(0, HALF)), (SIN, slice(0, HALF)),
                       (COS, slice(HALF, HD)), (SIN, slice(HALF, HD))):
        blocks.append(base * qscale[None, half])
        blocks.append(base * kscale[None, half])
    return np.concatenate(blocks, axis=1).astype(np.float32)


def _host_masks():
    # mask[r, c] = 1 if c >= r  (S.T block: rows k, cols q)
    r = np.arange(128)[:, None]
    c = np.arange(128)[None, :]
    return (c >= r).astype(np.float32)


def kernel(**inputs):
    from concourse.bass_utils import run_bass_kernel_spmd

    nc = _get_nc()
    bf16 = _np_bf16()

    x = np.asarray(inputs["inputs_q"], dtype=np.float32)
    wq = np.asarray(inputs["wq"], dtype=np.float32)
    wk = np.asarray(inputs["wk"], dtype=np.float32)
    wv = np.asarray(inputs["wv"], dtype=np.float32)
    wo = np.asarray(inputs["wo"], dtype=np.float32)

    ctab = _host_tables(np.asarray(inputs["q_ln_scale"], np.float32),
                        np.asarray(inputs["k_ln_scale"], np.float32),
                        np.asarray(inputs["per_dim_scale"], np.float32))
    ctab = ctab.astype(bf16)
    masks = _host_masks().astype(bf16)

    in_maps = []
    for c in range(8):
        b, g = divmod(c, 4)
        hs = slice(NH * g, NH * (g + 1))
        wqkv_c = np.concatenate(
            [wq[:, hs, :].reshape(D, NH * HD),
             wk[:, hs, :].reshape(D, NH * HD),
             wv[:, hs, :].reshape(D, NH * HD)], axis=1)
        in_maps.append({
            "x": np.ascontiguousarray(x[b].T).astype(bf16),
            "wqkv": np.ascontiguousarray(wqkv_c).astype(bf16),
            "wo": np.ascontiguousarray(wo[hs].reshape(NH * HD, D)).astype(bf16),
            "ctab": ctab, "masks": masks,
            "ident": np.eye(128, dtype=bf16),
        })

    trace = os.environ.get("MHA_TRACE", "0") == "1"
    res = run_bass_kernel_spmd(nc, in_maps, list(range(8)), trace=trace)
    if trace:
        kernel.last_exec_time_ns = res.exec_time_ns
        kernel.last_results = res

    out = np.zeros((B, N, D), dtype=np.float32)
    for c in range(8):
        out[c // 4] += res.results[c]["outT"].T
    return out


# revision 20
# speedup vs baseline: 1.3467x; 1.3467x over previous
"""Trainium2 Bass kernel for nn_MultiHeadAttention_68152541053005.

Multi-head attention (B=2, N=2048, D=1024, H=16, d=64) with RoPE,
per-head RMSNorm on q/k, per-dim scale on q, causal softmax.

Sharding: 8 cores = 2 batch groups x 4 head-groups (4 heads/core).
Each core computes QKV projection for its 4 heads on its batch,
attention, and a partial output projection; the host sums the 4
partial outputs per batch (equivalent to the all-reduce after the
output projection).

Per-core kernel, software-pipelined across engines:
  - single interleaved instruction stream: QKV projection for token
    group g+1 is woven between attention matmuls for q-block g, so the
    PE never idles while ACT runs softmax exp (and stays at its max
    p-state, which needs ~3us of continuous PE work)
  - qT/kT produced by DMA XBAR transposes (no PE transposes)
  - ACT does exp + qk psum eviction only; DVE does psum-touching ops
    (v evict, reduce, rsqrt, recip, normalize, outproj evict); the
    Pool engine does sbuf-only elementwise (RoPE, squares, per-dim
    scale, causal masks)
  - softmax denominator rides as a ones-row in the ctx matmul; its
    reciprocal is partition-broadcast via a K=1 PE matmul
  - PSUM plan (8 banks): pqkv[128,768]f32 x2 bufs (4) + st[128,1024]
    f32 x1 (2) + ctx[128,512] x1 (1) + misc(po/bcast)[128,512] x1 (1)
"""

import os
import sys

if "/opt/trn_rl_repo" not in sys.path:
    sys.path.insert(0, "/opt/trn_rl_repo")

import numpy as np
from contextlib import ExitStack

import concourse.bacc as bacc
import concourse.bass as bass
import concourse.mybir as mybir
import concourse.tile as tile

AP = bass.AP
F32 = mybir.dt.float32
BF16 = mybir.dt.bfloat16
AFT = mybir.ActivationFunctionType
ALU = mybir.AluOpType

B, N, D, H, HD = 2, 2048, 1024, 16, 64
NH = 4            # heads per core
HALF = HD // 2    # 32
TC = N // 128     # 16 token chunks
DC = D // 128     # 8 D chunks
QB = N // 512     # 4 q blocks
LOG2_E = 1.442695041
RMS_EPS = 1e-6
MAX_TIMESCALE = 10000.0

# rsqrt(v) on DVE: z0 = c2*(v+h)^2 + k, then 2 Newton steps
# z <- z*(1.5 - 0.5*v*z^2); max rel err 8.5e-5 on v in [0.3, 2.3]
RS_H = -2.0157414099271302
RS_K = 0.6774616747941173
RS_C2 = 0.34740916
RS_VLO, RS_VHI = 0.3, 2.3


def _np_bf16():
    import ml_dtypes
    return np.dtype(ml_dtypes.bfloat16)


def build_nc():
    nc = bacc.Bacc("TRN2", target_bir_lowering=False, debug=False)

    x_d = nc.dram_tensor("x", [D, N], BF16, kind="ExternalInput")
    wqkv_d = nc.dram_tensor("wqkv", [D, 3 * NH * HD], BF16, kind="ExternalInput")
    wo_d = nc.dram_tensor("wo", [NH * HD, D], BF16, kind="ExternalInput")
    ctab_d = nc.dram_tensor("ctab", [N, 8 * HALF], BF16, kind="ExternalInput")
    masks_d = nc.dram_tensor("masks", [128, 128], BF16, kind="ExternalInput")
    ident_d = nc.dram_tensor("ident", [128, 128], BF16, kind="ExternalInput")
    outT_d = nc.dram_tensor("outT", [D, N], F32, kind="ExternalOutput")

    with tile.TileContext(nc) as tc, ExitStack() as ctx:
        build_tile_kernel(ctx, tc, x_d.ap(), wqkv_d.ap(), wo_d.ap(),
                          ctab_d.ap(), masks_d.ap(), ident_d.ap(),
                          outT_d.ap())
    nc.compile()
    return nc


def build_tile_kernel(ctx, tc, x, wqkv, wo, ctab, masks, identD, outT):
    nc = tc.nc

    res = ctx.enter_context(tc.tile_pool(name="res", bufs=1))
    stream = ctx.enter_context(tc.tile_pool(name="stream", bufs=3))
    scratch = ctx.enter_context(tc.tile_pool(name="scratch", bufs=2))
    qkpool = ctx.enter_context(tc.tile_pool(name="qkpool", bufs=3))
    ptp = ctx.enter_context(tc.tile_pool(name="ptp", bufs=3))
    ropep = ctx.enter_context(tc.tile_pool(name="ropep", bufs=5))

    pq = ctx.enter_context(tc.tile_pool(name="pq", bufs=1, space="PSUM"))
    pS = ctx.enter_context(tc.tile_pool(name="pS", bufs=2, space="PSUM"))
    pC = ctx.enter_context(tc.tile_pool(name="pC", bufs=1, space="PSUM"))
    pM = ctx.enter_context(tc.tile_pool(name="pM", bufs=1, space="PSUM"))

    # ---- resident constants ----
    wqkv_sb = res.tile([128, DC * 768], BF16, tag="wqkv")
    for c in range(DC):
        nc.sync.dma_start(wqkv_sb[:, 768 * c:768 * (c + 1)],
                          wqkv[128 * c:128 * (c + 1), :])
    wo_sb = res.tile([128, 2 * D], BF16, tag="wo")
    for r in range(2):
        nc.sync.dma_start(wo_sb[:, D * r:D * (r + 1)],
                          wo[128 * r:128 * (r + 1), :])
    masks_sb = res.tile([128, 128], BF16, tag="masks")
    nc.sync.dma_start(masks_sb[:], masks[:])
    ident = res.tile([128, 128], BF16, tag="ident")
    nc.sync.dma_start(ident[:], identD[:])

    qkT_all = res.tile([128, 4 * N], BF16, tag="qkT_all")
    qT = [qkT_all[:, i * N:(i + 1) * N] for i in range(2)]
    kT = [qkT_all[:, (2 + i) * N:(3 + i) * N] for i in range(2)]
    ctxT = [res.tile([128, N], BF16, tag=f"ctxT{i}", name=f"ctxT{i}")
            for i in range(2)]
    vt = res.tile([128, TC * NH * 65], BF16, tag="vt")
    va = vt[:]
    ones_dst = AP(va.tensor, va.offset + HD,
                  [va.ap[0], [NH * 65, TC], [65, NH], [1, 1]])
    nc.vector.memset(ones_dst, 1.0)

    # x is [D, N] in DRAM; stage the token-chunk columns in waves so
    # early chunks unblock quickly
    xT_big = [res.tile([128, N], BF16, tag=f"xT{c}", name=f"xTbig{c}")
              for c in range(DC)]
    for lo, hi in ((0, 128), (128, 512), (512, 1024), (1024, 2048)):
        for c in range(DC):
            nc.sync.dma_start(xT_big[c][:, lo:hi],
                              x[128 * c:128 * (c + 1), lo:hi])

    # ------------------------------------------------------------------
    # chunk pipeline: QKV projection + postproc for token chunk t
    # ------------------------------------------------------------------
    group_state = {}

    def make_chunk_units(t):
        state = {}
        gi, dt_i = divmod(t, 4)

        def mk(c):
            def u():
                if c == 0:
                    ctab_t = stream.tile([128, 256], BF16, tag="ctab",
                                         name=f"ctab{t}")
                    nc.sync.dma_start(ctab_t[:],
                                      ctab[128 * t:128 * (t + 1), :])
                    state["ctab"] = ctab_t
                    state["pqkv"] = pq.tile([128, 768], F32, tag="pqkv",
                                            name=f"pqkv{t}")
                pqkv = state["pqkv"]
                lhsT = xT_big[c][:, 128 * t:128 * (t + 1)]
                nc.tensor.matmul(pqkv[:, 0:512],
                                 lhsT, wqkv_sb[:, 768 * c:768 * c + 512],
                                 start=(c == 0), stop=(c == DC - 1))
                nc.tensor.matmul(pqkv[:, 512:768],
                                 lhsT, wqkv_sb[:, 768 * c + 512:768 * (c + 1)],
                                 start=(c == 0), stop=(c == DC - 1))
                if c == DC - 1:
                    post(state["pqkv"], state["ctab"])
            return (768, u)

        def post(pqkv, ctab_t):
            # qk eviction (ACT), v eviction (DVE, strided with ones gaps)
            qk_sb = qkpool.tile([128, 512], BF16, tag="qk_sb",
                                name=f"qk_sb{t}")
            nc.scalar.copy(qk_sb[:], pqkv[:, 0:512])
            va2 = vt[:]
            v_dst = AP(va2.tensor, va2.offset + NH * 65 * t,
                       [va2.ap[0], [65, NH], [1, HD]])
            pa = pqkv[:]
            v_src = AP(pa.tensor, pa.offset + 512,
                       [pa.ap[0], [HD, NH], [1, HD]])
            nc.vector.tensor_copy(v_dst, v_src)

            # RMSNorm stats from pre-RoPE q/k (rotation preserves norms);
            # reduce into the group's [128, 32] stats tile
            sq = scratch.tile([128, 512], BF16, tag="sq", name=f"sq{t}")
            nc.vector.tensor_mul(sq[:], qk_sb[:], qk_sb[:])
            if dt_i == 0:
                group_state[gi] = {
                    "ssq": scratch.tile([128, 32], F32, tag="ssq",
                                        name=f"ssq_g{gi}")
                }
            ssq = group_state[gi]["ssq"]
            nc.vector.reduce_sum(ssq[:, 8 * dt_i:8 * (dt_i + 1)],
                                 sq[:].rearrange("p (h d) -> p h d", d=HD),
                                 axis=mybir.AxisListType.X)

            # RoPE on DVE via host-folded tables (per-dim scales folded in)
            def dat(off, tl):
                a = tl[:]
                return AP(a.tensor, a.offset + off,
                          [a.ap[0], [256, 2], [HD, NH], [1, HALF]])

            def tab(f):
                a = ctab_t[:]
                return AP(a.tensor, a.offset + 64 * f,
                          [a.ap[0], [HALF, 2], [0, NH], [1, HALF]])

            tmp = [scratch.tile([128, 256], BF16, tag=f"rp{i}",
                                name=f"rp{i}_{t}") for i in range(4)]
            roped = ropep.tile([128, 512], BF16, tag="roped",
                               name=f"roped{t}")
            nc.gpsimd.tensor_mul(tmp[0][:], dat(0, qk_sb), tab(0))
            nc.gpsimd.tensor_mul(tmp[1][:], dat(HALF, qk_sb), tab(1))
            nc.gpsimd.tensor_sub(dat(0, roped), tmp[0][:], tmp[1][:])
            nc.gpsimd.tensor_mul(tmp[2][:], dat(HALF, qk_sb), tab(2))
            nc.gpsimd.tensor_mul(tmp[3][:], dat(0, qk_sb), tab(3))
            nc.gpsimd.tensor_add(dat(HALF, roped), tmp[2][:], tmp[3][:])
            group_state[gi][f"roped{dt_i}"] = roped

            if dt_i == 3:
                # batched rsqrt for the group (quadratic seed + 2 Newton)
                v_ = scratch.tile([128, 32], F32, tag="rsv", name=f"rsv{gi}")
                nc.vector.tensor_scalar(v_[:], ssq[:], 1.0 / HD, RMS_EPS,
                                        ALU.mult, ALU.add)
                vc = scratch.tile([128, 32], F32, tag="rsvc",
                                  name=f"rsvc{gi}")
                nc.vector.tensor_scalar(vc[:], v_[:], RS_VLO, RS_VHI,
                                        ALU.max, ALU.min)
                t_ = scratch.tile([128, 32], F32, tag="rst", name=f"rst{gi}")
                nc.vector.tensor_scalar_add(t_[:], vc[:], RS_H)
                z_ = scratch.tile([128, 32], F32, tag="rsz", name=f"rsz{gi}")
                nc.vector.scalar_tensor_tensor(z_[:], t_[:], RS_C2, t_[:],
                                               ALU.mult, ALU.mult)
                nc.vector.tensor_scalar_add(z_[:], z_[:], RS_K)
                z2 = scratch.tile([128, 32], F32, tag="rsz2",
                                  name=f"rsz2{gi}")
                w_ = scratch.tile([128, 32], F32, tag="rsw", name=f"rsw{gi}")
                rs = scratch.tile([128, 32], F32, tag="rs_g", name=f"rs{gi}")
                for it in range(2):
                    nc.vector.tensor_mul(z2[:], z_[:], z_[:])
                    nc.vector.scalar_tensor_tensor(w_[:], z2[:], -0.5, v_[:],
                                                   ALU.mult, ALU.mult)
                    out_ = rs if it == 1 else z_
                    nc.vector.scalar_tensor_tensor(out_[:], w_[:], 1.5, z_[:],
                                                   ALU.add, ALU.mult)
                group_state[gi]["rs"] = rs

        return [mk(c) for c in range(DC)]

    def make_trans_unit(t):
        gi, dt_i = divmod(t, 4)

        def u():
            roped = group_state[gi].pop(f"roped{dt_i}")
            rs = group_state[gi]["rs"]
            # apply per-(token, head) rsqrt; broadcast over the 64 dims
            qk_stage = scratch.tile([128, 512], BF16, tag="qk_stage",
                                    name=f"qk_stage{t}")
            ra = rs[:]
            rs_b = AP(ra.tensor, ra.offset + 8 * dt_i,
                      [ra.ap[0], [1, 8], [0, HD]])
            nc.vector.tensor_mul(
                qk_stage[:].rearrange("p (h d) -> p h d", d=HD),
                roped[:].rearrange("p (h d) -> p h d", d=HD), rs_b)
            ptq = pM.tile([128, 512], BF16, tag="misc", name=f"ptq{t}")
            for i in range(4):
                nc.tensor.transpose(ptq[:, 128 * i:128 * (i + 1)],
                                    qk_stage[:, 128 * i:128 * (i + 1)],
                                    ident[:])
            qa_ = qkT_all[:]
            dst = AP(qa_.tensor, qa_.offset + 128 * t,
                     [qa_.ap[0], [N, 4], [1, 128]])
            nc.vector.tensor_copy(dst, ptq[:])
        return (512, u)

    # ------------------------------------------------------------------
    # attention for q block Q (512 queries), head h
    # ------------------------------------------------------------------
    def make_attn_units(Q):
        qcol = slice(512 * Q, 512 * (Q + 1))
        units = []
        q_state = {}
        for h in range(NH):
            g, off = divmod(h, 2)
            row = slice(64 * off, 64 * off + 64)
            npair = 2 * Q + 2
            st_state = {}

            def mk_st(p, g=g, row=row, h=h):
                def u():
                    pst = pS.tile([128, 1024], F32, tag="st",
                                  name=f"st{Q}_{h}_{p}")
                    regions = []
                    for s in range(2):
                        j = 2 * p + s
                        qoff = max(0, 128 * j - 512 * Q)
                        cols = 512 - qoff
                        nc.tensor.matmul(
                            pst[:, 512 * s:512 * s + cols],
                            kT[g][row, 128 * j:128 * (j + 1)],
                            qT[g][row, 512 * Q + qoff:512 * (Q + 1)],
                            start=True, stop=True)
                        regions.append(cols)
                    pt = ptp.tile([128, 1024], BF16, tag="pt",
                                  name=f"pt{Q}_{h}_{p}")
                    c0, c1 = regions
                    if c0 == 512:
                        nc.scalar.activation(pt[:, 0:512 + c1],
                                             pst[:, 0:512 + c1], AFT.Exp)
                    else:
                        nc.scalar.activation(pt[:, 0:c0], pst[:, 0:c0],
                                             AFT.Exp)
                        nc.scalar.activation(pt[:, 512:512 + c1],
                                             pst[:, 512:512 + c1], AFT.Exp)
                    if p >= 2 * Q:
                        nc.vector.tensor_mul(pt[:, 0:128], pt[:, 0:128],
                                             masks_sb[:])
                        nc.vector.tensor_mul(pt[:, 512:640], pt[:, 512:640],
                                             masks_sb[:])
                    st_state[p] = pt
                cost = sum(512 - max(0, 128 * (2 * p + s) - 512 * Q)
                           for s in range(2))
                return (cost, u)

            def mk_ctx(p, h=h, npair=npair):
                def u():
                    if p == 0:
                        st_state["pctx"] = pC.tile([65, 512], F32, tag="ctx",
                                                   name=f"ctx{Q}_{h}")
                    pctx = st_state["pctx"]
                    pt = st_state.pop(p)
                    for s in range(2):
                        j = 2 * p + s
                        qoff = max(0, 128 * j - 512 * Q)
                        cols = 512 - qoff
                        nc.tensor.matmul(
                            pctx[:, qoff:512],
                            vt[:, 65 * (NH * j + h):65 * (NH * j + h) + 65],
                            pt[:, 512 * s:512 * s + cols],
                            start=(j == 0), stop=(j == 2 * npair - 1))
                cost = sum(512 - max(0, 128 * (2 * p + s) - 512 * Q)
                           for s in range(2))
                return (cost, u)

            def mk_evict(h=h):
                def u():
                    pctx = st_state.pop("pctx")
                    # evict raw ctx + den to SBUF fast to free the psum
                    # bank; the actual normalize is batched per q block
                    if h == 0:
                        q_state["u"] = scratch.tile(
                            [64, 4 * 512], F32, tag="u_sb",
                            name=f"u{Q}", bufs=2)
                        q_state["den"] = scratch.tile(
                            [1, 4 * 512], F32, tag="den_sb",
                            name=f"den{Q}", bufs=2)
                    nc.vector.tensor_copy(
                        q_state["u"][:, 512 * h:512 * (h + 1)], pctx[0:64, :])
                    nc.vector.tensor_copy(
                        q_state["den"][:, 512 * h:512 * (h + 1)],
                        pctx[64:65, :])
                return (0, u)

            units.append(mk_st(0))
            for p in range(1, npair):
                units.append(mk_st(p))
                units.append(mk_ctx(p - 1))
            units.append(mk_ctx(npair - 1))
            units.append(mk_evict())

        def mk_norm_all():
            def u():
                u_sb, den_sb = q_state.pop("u"), q_state.pop("den")
                recip0 = scratch.tile([1, 4 * 512], F32, tag="recip0",
                                      name=f"recip{Q}")
                nc.vector.reciprocal_approx_fast(recip0[:], den_sb[:])
                bcast = scratch.tile([64, 4 * 512], F32, tag="bcast",
                                     name=f"bcast{Q}")
                nc.gpsimd.partition_broadcast(bcast[:], recip0[:])
                for h in range(NH):
                    g, off = divmod(h, 2)
                    row = slice(64 * off, 64 * off + 64)
                    nc.vector.tensor_mul(ctxT[g][row, qcol],
                                         u_sb[:, 512 * h:512 * (h + 1)],
                                         bcast[:, 512 * h:512 * (h + 1)])
            return (0, u)

        units.append(mk_norm_all())
        return units

    # ------------------------------------------------------------------
    # output projection for q block Q
    # ------------------------------------------------------------------
    def make_outproj_units(Q):
        qcol = slice(512 * Q, 512 * (Q + 1))
        units = []
        for m in range(DC):
            def u(m=m):
                po = pM.tile([128, 512], F32, tag="misc",
                             name=f"po{Q}_{m}")
                for r in range(2):
                    nc.tensor.matmul(
                        po[:],
                        wo_sb[:, D * r + 128 * m:D * r + 128 * (m + 1)],
                        ctxT[r][:, qcol], start=(r == 0), stop=(r == 1))
                ob = scratch.tile([128, 512], F32, tag="ob",
                                  name=f"ob{Q}_{m}")
                if m % 2 == 0:
                    nc.scalar.copy(ob[:], po[:])
                else:
                    nc.vector.tensor_copy(ob[:], po[:])
                nc.sync.dma_start(outT[128 * m:128 * (m + 1), qcol], ob[:])
            units.append((1024, u))
        return units

    # ------------------------------------------------------------------
    # weave two unit streams proportionally by PE cost
    # ------------------------------------------------------------------
    def weave(A, B):
        totA = sum(c for c, _ in A) or 1
        totB = sum(c for c, _ in B) or 1
        out = []
        ia = ib = 0
        ca = cb = 0
        while ia < len(A) or ib < len(B):
            fa = ca / totA
            fb = cb / totB
            if ib >= len(B) or (ia < len(A) and fa <= fb):
                c, u = A[ia]; ia += 1; ca += c
            else:
                c, u = B[ib]; ib += 1; cb += c
            out.append(u)
        return out

    def make_group_units(g):
        units = []
        for t in range(4 * g, 4 * (g + 1)):
            units += make_chunk_units(t)
        for t in range(4 * g, 4 * (g + 1)):
            units.append(make_trans_unit(t))
        return units

    plan = []
    # group 0 alone (attention needs its k/v first)
    plan += [u for _, u in make_group_units(0)]
    # attn(Q) ∥ chunks of group Q+1 ∥ outproj(Q-1)
    for Q in range(3):
        other = make_attn_units(Q) + (make_outproj_units(Q - 1) if Q else [])
        plan += weave(other, make_group_units(Q + 1))
    plan += weave(make_attn_units(3), make_outproj_units(2))
    plan += [u for _, u in make_outproj_units(3)]

    for u in plan:
        u()


# ---------------------------------------------------------------------------
# host side
# ---------------------------------------------------------------------------

_CACHE = {}


def _get_nc():
    if "nc" not in _CACHE:
        _CACHE["nc"] = build_nc()
    return _CACHE["nc"]


def _host_tables(q_ln_scale, k_ln_scale, per_dim_scale):
    frac = 2.0 * np.arange(HALF, dtype=np.float32) / HD
    ts = (MAX_TIMESCALE ** frac).astype(np.float32)
    pos = np.arange(N, dtype=np.float32)
    sinu = pos[:, None] / ts[None, :]
    SIN = np.sin(sinu).astype(np.float32)
    COS = np.cos(sinu).astype(np.float32)
    qs = (LOG2_E / np.sqrt(np.float32(HD))
          * np.logaddexp(0.0, per_dim_scale.astype(np.float64))).astype(np.float32)
    qscale = (q_ln_scale * qs).astype(np.float32)
    kscale = k_ln_scale.astype(np.float32)

    # combined table [N, 256]: func f in {cosA,sinA,cosB,sinB} at cols
    # [64f:64f+64], q-scaled half at +0:32, k-scaled at +32:64
    blocks = []
    for base, half in ((COS, slice(0, HALF)), (SIN, slice(0, HALF)),
                       (COS, slice(HALF, HD)), (SIN, slice(HALF, HD))):
        blocks.append(base * qscale[None, half])
        blocks.append(base * kscale[None, half])
    return np.concatenate(blocks, axis=1).astype(np.float32)


def _host_masks():
    # mask[r, c] = 1 if c >= r  (S.T block: rows k, cols q)
    r = np.arange(128)[:, None]
    c = np.arange(128)[None, :]
    return (c >= r).astype(np.float32)


def kernel(**inputs):
    from concourse.bass_utils import run_bass_kernel_spmd

    nc = _get_nc()
    bf16 = _np_bf16()

    x = np.asarray(inputs["inputs_q"], dtype=np.float32)
    wq = np.asarray(inputs["wq"], dtype=np.float32)
    wk = np.asarray(inputs["wk"], dtype=np.float32)
    wv = np.asarray(inputs["wv"], dtype=np.float32)
    wo = np.asarray(inputs["wo"], dtype=np.float32)

    ctab = _host_tables(np.asarray(inputs["q_ln_scale"], np.float32),
                        np.asarray(inputs["k_ln_scale"], np.float32),
                        np.asarray(inputs["per_dim_scale"], np.float32))
    ctab = ctab.astype(bf16)
    masks = _host_masks().astype(bf16)

    in_maps = []
    for c in range(8):
        b, g = divmod(c, 4)
        hs = slice(NH * g, NH * (g + 1))
        wqkv_c = np.concatenate(
            [wq[:, hs, :].reshape(D, NH * HD),
             wk[:, hs, :].reshape(D, NH * HD),
             wv[:, hs, :].reshape(D, NH * HD)], axis=1)
        in_maps.append({
            "x": np.ascontiguousarray(x[b].T).astype(bf16),
            "wqkv": np.ascontiguousarray(wqkv_c).astype(bf16),
            "wo": np.ascontiguousarray(wo[hs].reshape(NH * HD, D)).astype(bf16),
            "ctab": ctab, "masks": masks,
            "ident": np.eye(128, dtype=bf16),
        })

    trace = os.environ.get("MHA_TRACE", "0") == "1"
    res = run_bass_kernel_spmd(nc, in_maps, list(range(8)), trace=trace)
    if trace:
        kernel.last_exec_time_ns = res.exec_time_ns
        kernel.last_results = res

    out = np.zeros((B, N, D), dtype=np.float32)
    for c in range(8):
        out[c // 4] += res.results[c]["outT"].T
    return out


# revision 22
# speedup vs baseline: 1.4405x; 1.0696x over previous
"""Trainium2 Bass kernel for nn_MultiHeadAttention_68152541053005.

Multi-head attention (B=2, N=2048, D=1024, H=16, d=64) with RoPE,
per-head RMSNorm on q/k, per-dim scale on q, causal softmax.

Sharding: 8 cores = 2 batch groups x 4 head-groups (4 heads/core).
Each core computes QKV projection for its 4 heads on its batch,
attention, and a partial output projection; the host sums the 4
partial outputs per batch (equivalent to the all-reduce after the
output projection).

Per-core kernel, software-pipelined across engines:
  - single interleaved instruction stream: QKV projection for token
    group g+1 is woven between attention matmuls for q-block g, so the
    PE never idles while ACT runs softmax exp (and stays at its max
    p-state, which needs ~3us of continuous PE work)
  - qT/kT produced by DMA XBAR transposes (no PE transposes)
  - ACT does exp + qk psum eviction only; DVE does psum-touching ops
    (v evict, reduce, rsqrt, recip, normalize, outproj evict); the
    Pool engine does sbuf-only elementwise (RoPE, squares, per-dim
    scale, causal masks)
  - softmax denominator rides as a ones-row in the ctx matmul; its
    reciprocal is partition-broadcast via a K=1 PE matmul
  - PSUM plan (8 banks): pqkv[128,768]f32 x2 bufs (4) + st[128,1024]
    f32 x1 (2) + ctx[128,512] x1 (1) + misc(po/bcast)[128,512] x1 (1)
"""

import os
import sys

if "/opt/trn_rl_repo" not in sys.path:
    sys.path.insert(0, "/opt/trn_rl_repo")

import numpy as np
from contextlib import ExitStack

import concourse.bacc as bacc
import concourse.bass as bass
import concourse.mybir as mybir
import concourse.tile as tile

AP = bass.AP
F32 = mybir.dt.float32
BF16 = mybir.dt.bfloat16
AFT = mybir.ActivationFunctionType
ALU = mybir.AluOpType

B, N, D, H, HD = 2, 2048, 1024, 16, 64
NH = 4            # heads per core
HALF = HD // 2    # 32
TC = N // 128     # 16 token chunks
DC = D // 128     # 8 D chunks
QB = N // 512     # 4 q blocks
LOG2_E = 1.442695041
RMS_EPS = 1e-6
MAX_TIMESCALE = 10000.0

# rsqrt(v) on DVE: z0 = c2*(v+h)^2 + k, then 2 Newton steps
# z <- z*(1.5 - 0.5*v*z^2); max rel err 8.5e-5 on v in [0.3, 2.3]
RS_H = -2.0157414099271302
RS_K = 0.6774616747941173
RS_C2 = 0.34740916
RS_VLO, RS_VHI = 0.3, 2.3


def _np_bf16():
    import ml_dtypes
    return np.dtype(ml_dtypes.bfloat16)


def build_nc():
    nc = bacc.Bacc("TRN2", target_bir_lowering=False, debug=False)

    x_d = nc.dram_tensor("x", [D, N], BF16, kind="ExternalInput")
    wqkv_d = nc.dram_tensor("wqkv", [D, 3 * NH * HD], BF16, kind="ExternalInput")
    wo_d = nc.dram_tensor("wo", [NH * HD, D], BF16, kind="ExternalInput")
    ctab_d = nc.dram_tensor("ctab", [N, 8 * HALF], BF16, kind="ExternalInput")
    masks_d = nc.dram_tensor("masks", [128, 128], BF16, kind="ExternalInput")
    ident_d = nc.dram_tensor("ident", [128, 128], BF16, kind="ExternalInput")
    outT_d = nc.dram_tensor("outT", [D, N], F32, kind="ExternalOutput")

    with tile.TileContext(nc) as tc, ExitStack() as ctx:
        build_tile_kernel(ctx, tc, x_d.ap(), wqkv_d.ap(), wo_d.ap(),
                          ctab_d.ap(), masks_d.ap(), ident_d.ap(),
                          outT_d.ap())
    nc.compile()
    return nc


def build_tile_kernel(ctx, tc, x, wqkv, wo, ctab, masks, identD, outT):
    nc = tc.nc

    res = ctx.enter_context(tc.tile_pool(name="res", bufs=1))
    stream = ctx.enter_context(tc.tile_pool(name="stream", bufs=3))
    scratch = ctx.enter_context(tc.tile_pool(name="scratch", bufs=2))
    qkpool = ctx.enter_context(tc.tile_pool(name="qkpool", bufs=3))
    ptp = ctx.enter_context(tc.tile_pool(name="ptp", bufs=3))
    ropep = ctx.enter_context(tc.tile_pool(name="ropep", bufs=5))

    drp = ctx.enter_context(tc.tile_pool(name="drp", bufs=2, space="DRAM"))
    pq = ctx.enter_context(tc.tile_pool(name="pq", bufs=1, space="PSUM"))
    pS = ctx.enter_context(tc.tile_pool(name="pS", bufs=2, space="PSUM"))
    pC = ctx.enter_context(tc.tile_pool(name="pC", bufs=1, space="PSUM"))
    pM = ctx.enter_context(tc.tile_pool(name="pM", bufs=1, space="PSUM"))

    # ---- resident constants ----
    wqkv_sb = res.tile([128, DC * 768], BF16, tag="wqkv")
    for c in range(DC):
        nc.sync.dma_start(wqkv_sb[:, 768 * c:768 * (c + 1)],
                          wqkv[128 * c:128 * (c + 1), :])
    wo_sb = res.tile([128, 2 * D], BF16, tag="wo")
    for r in range(2):
        nc.sync.dma_start(wo_sb[:, D * r:D * (r + 1)],
                          wo[128 * r:128 * (r + 1), :])
    masks_sb = res.tile([128, 128], BF16, tag="masks")
    nc.sync.dma_start(masks_sb[:], masks[:])
    ident = res.tile([128, 128], BF16, tag="ident")
    nc.sync.dma_start(ident[:], identD[:])

    qkT_all = res.tile([128, 4 * N], BF16, tag="qkT_all")
    qT = [qkT_all[:, i * N:(i + 1) * N] for i in range(2)]
    kT = [qkT_all[:, (2 + i) * N:(3 + i) * N] for i in range(2)]
    ctxT = [res.tile([128, N], BF16, tag=f"ctxT{i}", name=f"ctxT{i}")
            for i in range(2)]
    vt = res.tile([128, TC * NH * 65], BF16, tag="vt")
    va = vt[:]
    ones_dst = AP(va.tensor, va.offset + HD,
                  [va.ap[0], [NH * 65, TC], [65, NH], [1, 1]])
    nc.vector.memset(ones_dst, 1.0)

    # x is [D, N] in DRAM; stage the token-chunk columns in waves so
    # early chunks unblock quickly
    xT_big = [res.tile([128, N], BF16, tag=f"xT{c}", name=f"xTbig{c}")
              for c in range(DC)]
    for lo, hi in ((0, 128), (128, 512), (512, 1024), (1024, 2048)):
        for c in range(DC):
            nc.sync.dma_start(xT_big[c][:, lo:hi],
                              x[128 * c:128 * (c + 1), lo:hi])

    # ------------------------------------------------------------------
    # chunk pipeline: QKV projection + postproc for token chunk t
    # ------------------------------------------------------------------
    group_state = {}

    def make_chunk_units(t):
        state = {}
        gi, dt_i = divmod(t, 4)

        def mk(c):
            def u():
                if c == 0:
                    ctab_t = stream.tile([128, 256], BF16, tag="ctab",
                                         name=f"ctab{t}")
                    nc.sync.dma_start(ctab_t[:],
                                      ctab[128 * t:128 * (t + 1), :])
                    state["ctab"] = ctab_t
                    state["pqkv"] = pq.tile([128, 768], F32, tag="pqkv",
                                            name=f"pqkv{t}")
                pqkv = state["pqkv"]
                lhsT = xT_big[c][:, 128 * t:128 * (t + 1)]
                nc.tensor.matmul(pqkv[:, 0:512],
                                 lhsT, wqkv_sb[:, 768 * c:768 * c + 512],
                                 start=(c == 0), stop=(c == DC - 1))
                nc.tensor.matmul(pqkv[:, 512:768],
                                 lhsT, wqkv_sb[:, 768 * c + 512:768 * (c + 1)],
                                 start=(c == 0), stop=(c == DC - 1))
                if c == DC - 1:
                    post(state["pqkv"], state["ctab"])
            return (768, u)

        def post(pqkv, ctab_t):
            # qk eviction (ACT), v eviction (DVE, strided with ones gaps)
            qk_sb = qkpool.tile([128, 512], BF16, tag="qk_sb",
                                name=f"qk_sb{t}")
            nc.scalar.copy(qk_sb[:], pqkv[:, 0:512])
            va2 = vt[:]
            v_dst = AP(va2.tensor, va2.offset + NH * 65 * t,
                       [va2.ap[0], [65, NH], [1, HD]])
            pa = pqkv[:]
            v_src = AP(pa.tensor, pa.offset + 512,
                       [pa.ap[0], [HD, NH], [1, HD]])
            nc.vector.tensor_copy(v_dst, v_src)

            # RMSNorm stats from pre-RoPE q/k (rotation preserves norms);
            # reduce into the group's [128, 32] stats tile
            sq = scratch.tile([128, 512], BF16, tag="sq", name=f"sq{t}")
            nc.vector.tensor_mul(sq[:], qk_sb[:], qk_sb[:])
            if dt_i == 0:
                group_state[gi] = {
                    "ssq": scratch.tile([128, 32], F32, tag="ssq",
                                        name=f"ssq_g{gi}")
                }
            ssq = group_state[gi]["ssq"]
            nc.vector.reduce_sum(ssq[:, 8 * dt_i:8 * (dt_i + 1)],
                                 sq[:].rearrange("p (h d) -> p h d", d=HD),
                                 axis=mybir.AxisListType.X)

            # RoPE on DVE via host-folded tables (per-dim scales folded in)
            def dat(off, tl):
                a = tl[:]
                return AP(a.tensor, a.offset + off,
                          [a.ap[0], [256, 2], [HD, NH], [1, HALF]])

            def tab(f):
                a = ctab_t[:]
                return AP(a.tensor, a.offset + 64 * f,
                          [a.ap[0], [HALF, 2], [0, NH], [1, HALF]])

            tmp = [scratch.tile([128, 256], BF16, tag=f"rp{i}",
                                name=f"rp{i}_{t}") for i in range(4)]
            roped = ropep.tile([128, 512], BF16, tag="roped",
                               name=f"roped{t}")
            eng = nc.vector if dt_i == 3 else nc.gpsimd
            eng.tensor_mul(tmp[0][:], dat(0, qk_sb), tab(0))
            eng.tensor_mul(tmp[1][:], dat(HALF, qk_sb), tab(1))
            eng.tensor_sub(dat(0, roped), tmp[0][:], tmp[1][:])
            eng.tensor_mul(tmp[2][:], dat(HALF, qk_sb), tab(2))
            eng.tensor_mul(tmp[3][:], dat(0, qk_sb), tab(3))
            eng.tensor_add(dat(HALF, roped), tmp[2][:], tmp[3][:])
            group_state[gi][f"roped{dt_i}"] = roped

            if dt_i >= 2:
                # rsqrt (quadratic seed + 2 Newton steps); chunks 0-2
                # batch at dt 2 so their transposes aren't gated by
                # chunk 3; chunk 3 runs its own small chain
                lo, ncol = (0, 24) if dt_i == 2 else (24, 8)
                sl = slice(lo, lo + ncol)
                v_ = scratch.tile([128, 32], F32, tag="rsv",
                                  name=f"rsv{gi}_{dt_i}")
                nc.vector.tensor_scalar(v_[:, sl], ssq[:, sl], 1.0 / HD,
                                        RMS_EPS, ALU.mult, ALU.add)
                vc = scratch.tile([128, 32], F32, tag="rsvc",
                                  name=f"rsvc{gi}_{dt_i}")
                nc.vector.tensor_scalar(vc[:, sl], v_[:, sl], RS_VLO,
                                        RS_VHI, ALU.max, ALU.min)
                t_ = scratch.tile([128, 32], F32, tag="rst",
                                  name=f"rst{gi}_{dt_i}")
                nc.vector.tensor_scalar_add(t_[:, sl], vc[:, sl], RS_H)
                z_ = scratch.tile([128, 32], F32, tag="rsz",
                                  name=f"rsz{gi}_{dt_i}")
                nc.vector.scalar_tensor_tensor(z_[:, sl], t_[:, sl], RS_C2,
                                               t_[:, sl], ALU.mult, ALU.mult)
                nc.vector.tensor_scalar_add(z_[:, sl], z_[:, sl], RS_K)
                z2 = scratch.tile([128, 32], F32, tag="rsz2",
                                  name=f"rsz2{gi}_{dt_i}")
                w_ = scratch.tile([128, 32], F32, tag="rsw",
                                  name=f"rsw{gi}_{dt_i}")
                if dt_i == 2:
                    rs = scratch.tile([128, 32], F32, tag="rs_g",
                                      name=f"rs{gi}")
                    group_state[gi]["rs"] = rs
                else:
                    rs = group_state[gi]["rs"]
                for it in range(2):
                    nc.vector.tensor_mul(z2[:, sl], z_[:, sl], z_[:, sl])
                    nc.vector.scalar_tensor_tensor(w_[:, sl], z2[:, sl],
                                                   -0.5, v_[:, sl],
                                                   ALU.mult, ALU.mult)
                    out_ = rs if it == 1 else z_
                    out_sl = out_[:, sl] if it == 1 else out_[:, sl]
                    nc.vector.scalar_tensor_tensor(out_sl, w_[:, sl], 1.5,
                                                   z_[:, sl],
                                                   ALU.add, ALU.mult)

        return [mk(c) for c in range(DC)]

    def make_trans_unit(t):
        gi, dt_i = divmod(t, 4)

        def u():
            roped = group_state[gi].pop(f"roped{dt_i}")
            rs = group_state[gi]["rs"]
            # apply per-(token, head) rsqrt; broadcast over the 64 dims
            qk_stage = scratch.tile([128, 512], BF16, tag="qk_stage",
                                    name=f"qk_stage{t}")
            ra = rs[:]
            rs_b = AP(ra.tensor, ra.offset + 8 * dt_i,
                      [ra.ap[0], [1, 8], [0, HD]])
            nc.vector.tensor_mul(
                qk_stage[:].rearrange("p (h d) -> p h d", d=HD),
                roped[:].rearrange("p (h d) -> p h d", d=HD), rs_b)
            ptq = pM.tile([128, 512], BF16, tag="misc", name=f"ptq{t}")
            for i in range(4):
                nc.tensor.transpose(ptq[:, 128 * i:128 * (i + 1)],
                                    qk_stage[:, 128 * i:128 * (i + 1)],
                                    ident[:])
            qa_ = qkT_all[:]
            dst = AP(qa_.tensor, qa_.offset + 128 * t,
                     [qa_.ap[0], [N, 4], [1, 128]])
            nc.vector.tensor_copy(dst, ptq[:])
        return (512, u)

    # ------------------------------------------------------------------
    # attention for q block Q (512 queries), head h
    # ------------------------------------------------------------------
    def make_attn_units(Q):
        qcol = slice(512 * Q, 512 * (Q + 1))
        units = []
        q_state = {}
        for h in range(NH):
            g, off = divmod(h, 2)
            row = slice(64 * off, 64 * off + 64)
            npair = 2 * Q + 2
            st_state = {}

            def mk_st(p, g=g, row=row, h=h):
                def u():
                    pst = pS.tile([128, 1024], F32, tag="st",
                                  name=f"st{Q}_{h}_{p}")
                    regions = []
                    for s in range(2):
                        j = 2 * p + s
                        qoff = max(0, 128 * j - 512 * Q)
                        cols = 512 - qoff
                        nc.tensor.matmul(
                            pst[:, 512 * s:512 * s + cols],
                            kT[g][row, 128 * j:128 * (j + 1)],
                            qT[g][row, 512 * Q + qoff:512 * (Q + 1)],
                            start=True, stop=True)
                        regions.append(cols)
                    pt = ptp.tile([128, 1024], BF16, tag="pt",
                                  name=f"pt{Q}_{h}_{p}")
                    c0, c1 = regions
                    if c0 == 512:
                        nc.scalar.activation(pt[:, 0:512 + c1],
                                             pst[:, 0:512 + c1], AFT.Exp)
                    else:
                        nc.scalar.activation(pt[:, 0:c0], pst[:, 0:c0],
                                             AFT.Exp)
                        nc.scalar.activation(pt[:, 512:512 + c1],
                                             pst[:, 512:512 + c1], AFT.Exp)
                    if p >= 2 * Q:
                        nc.vector.tensor_mul(pt[:, 0:128], pt[:, 0:128],
                                             masks_sb[:])
                        nc.vector.tensor_mul(pt[:, 512:640], pt[:, 512:640],
                                             masks_sb[:])
                    st_state[p] = pt
                cost = sum(512 - max(0, 128 * (2 * p + s) - 512 * Q)
                           for s in range(2))
                return (cost, u)

            def mk_ctx(p, h=h, npair=npair):
                def u():
                    if p == 0:
                        st_state["pctx"] = pC.tile([65, 512], F32, tag="ctx",
                                                   name=f"ctx{Q}_{h}")
                    pctx = st_state["pctx"]
                    pt = st_state.pop(p)
                    for s in range(2):
                        j = 2 * p + s
                        qoff = max(0, 128 * j - 512 * Q)
                        cols = 512 - qoff
                        nc.tensor.matmul(
                            pctx[:, qoff:512],
                            vt[:, 65 * (NH * j + h):65 * (NH * j + h) + 65],
                            pt[:, 512 * s:512 * s + cols],
                            start=(j == 0), stop=(j == 2 * npair - 1))
                cost = sum(512 - max(0, 128 * (2 * p + s) - 512 * Q)
                           for s in range(2))
                return (cost, u)

            def mk_evict(h=h):
                def u():
                    pctx = st_state.pop("pctx")
                    # evict raw ctx + den to SBUF fast to free the psum
                    # bank; the actual normalize is batched per q block
                    if h == 0:
                        q_state["u"] = scratch.tile(
                            [64, 4 * 512], F32, tag="u_sb",
                            name=f"u{Q}", bufs=2)
                        q_state["den"] = scratch.tile(
                            [1, 4 * 512], F32, tag="den_sb",
                            name=f"den{Q}", bufs=2)
                    nc.vector.tensor_copy(
                        q_state["u"][:, 512 * h:512 * (h + 1)], pctx[0:64, :])
                    nc.vector.tensor_copy(
                        q_state["den"][:, 512 * h:512 * (h + 1)],
                        pctx[64:65, :])
                return (0, u)

            units.append(mk_st(0))
            for p in range(1, npair):
                units.append(mk_st(p))
                units.append(mk_ctx(p - 1))
            units.append(mk_ctx(npair - 1))
            units.append(mk_evict())

        def mk_norm_all():
            def u():
                u_sb, den_sb = q_state.pop("u"), q_state.pop("den")
                recip0 = scratch.tile([1, 4 * 512], F32, tag="recip0",
                                      name=f"recip{Q}")
                nc.vector.reciprocal_approx_fast(recip0[:], den_sb[:])
                bcast = scratch.tile([64, 4 * 512], F32, tag="bcast",
                                     name=f"bcast{Q}")
                rb = drp.tile([1, 4 * 512], F32, tag="rb", name=f"rb{Q}")
                nc.sync.dma_start(rb[:], recip0[:])
                ra = rb[:]
                src_b = AP(ra.tensor, ra.offset, [[0, 64], [1, 4 * 512]])
                nc.sync.dma_start(bcast[:], src_b)
                for h in range(NH):
                    g, off = divmod(h, 2)
                    row = slice(64 * off, 64 * off + 64)
                    nc.vector.tensor_mul(ctxT[g][row, qcol],
                                         u_sb[:, 512 * h:512 * (h + 1)],
                                         bcast[:, 512 * h:512 * (h + 1)])
            return (0, u)

        units.append(mk_norm_all())
        return units

    # ------------------------------------------------------------------
    # output projection for q block Q
    # ------------------------------------------------------------------
    def make_outproj_units(Q):
        qcol = slice(512 * Q, 512 * (Q + 1))
        units = []
        for m in range(DC):
            def u(m=m):
                po = pM.tile([128, 512], F32, tag="misc",
                             name=f"po{Q}_{m}")
                for r in range(2):
                    nc.tensor.matmul(
                        po[:],
                        wo_sb[:, D * r + 128 * m:D * r + 128 * (m + 1)],
                        ctxT[r][:, qcol], start=(r == 0), stop=(r == 1))
                ob = scratch.tile([128, 512], F32, tag="ob",
                                  name=f"ob{Q}_{m}")
                if m % 2 == 0:
                    nc.scalar.copy(ob[:], po[:])
                else:
                    nc.vector.tensor_copy(ob[:], po[:])
                nc.sync.dma_start(outT[128 * m:128 * (m + 1), qcol], ob[:])
            units.append((1024, u))
        return units

    # ------------------------------------------------------------------
    # weave two unit streams proportionally by PE cost
    # ------------------------------------------------------------------
    def weave(A, B):
        totA = sum(c for c, _ in A) or 1
        totB = sum(c for c, _ in B) or 1
        out = []
        ia = ib = 0
        ca = cb = 0
        while ia < len(A) or ib < len(B):
            fa = ca / totA
            fb = cb / totB
            if ib >= len(B) or (ia < len(A) and fa <= fb):
                c, u = A[ia]; ia += 1; ca += c
            else:
                c, u = B[ib]; ib += 1; cb += c
            out.append(u)
        return out

    def make_group_units(g):
        units = []
        for t in range(4 * g, 4 * (g + 1)):
            units += make_chunk_units(t)
        for t in range(4 * g, 4 * (g + 1)):
            units.append(make_trans_unit(t))
        return units

    plan = []
    # group 0 alone (attention needs its k/v first)
    plan += [u for _, u in make_group_units(0)]
    # attn(Q) ∥ chunks of group Q+1 ∥ outproj(Q-1)
    for Q in range(3):
        other = make_attn_units(Q) + (make_outproj_units(Q - 1) if Q else [])
        plan += weave(other, make_group_units(Q + 1))
    plan += weave(make_attn_units(3), make_outproj_units(2))
    plan += [u for _, u in make_outproj_units(3)]

    for u in plan:
        u()


# ---------------------------------------------------------------------------
# host side
# ---------------------------------------------------------------------------

_CACHE = {}


def _get_nc():
    if "nc" not in _CACHE:
        _CACHE["nc"] = build_nc()
    return _CACHE["nc"]


def _host_tables(q_ln_scale, k_ln_scale, per_dim_scale):
    frac = 2.0 * np.arange(HALF, dtype=np.float32) / HD
    ts = (MAX_TIMESCALE ** frac).astype(np.float32)
    pos = np.arange(N, dtype=np.float32)
    sinu = pos[:, None] / ts[None, :]
    SIN = np.sin(sinu).astype(np.float32)
    COS = np.cos(sinu).astype(np.float32)
    qs = (LOG2_E / np.sqrt(np.float32(HD))
          * np.logaddexp(0.0, per_dim_scale.astype(np.float64))).astype(np.float32)
    qscale = (q_ln_scale * qs).astype(np.float32)
    kscale = k_ln_scale.astype(np.float32)

    # combined table [N, 256]: func f in {cosA,sinA,cosB,sinB} at cols
    # [64f:64f+64], q-scaled half at +0:32, k-scaled at +32:64
    blocks = []
    for base, half in ((COS, slice(0, HALF)), (SIN, slice(0, HALF)),
                       (COS, slice(HALF, HD)), (SIN, slice(HALF, HD))):
        blocks.append(base * qscale[None, half])
        blocks.append(base * kscale[None, half])
    return np.concatenate(blocks, axis=1).astype(np.float32)


def _host_masks():
    # mask[r, c] = 1 if c >= r  (S.T block: rows k, cols q)
    r = np.arange(128)[:, None]
    c = np.arange(128)[None, :]
    return (c >= r).astype(np.float32)


def kernel(**inputs):
    from concourse.bass_utils import run_bass_kernel_spmd

    nc = _get_nc()
    bf16 = _np_bf16()

    x = np.asarray(inputs["inputs_q"], dtype=np.float32)
    wq = np.asarray(inputs["wq"], dtype=np.float32)
    wk = np.asarray(inputs["wk"], dtype=np.float32)
    wv = np.asarray(inputs["wv"], dtype=np.float32)
    wo = np.asarray(inputs["wo"], dtype=np.float32)

    ctab = _host_tables(np.asarray(inputs["q_ln_scale"], np.float32),
                        np.asarray(inputs["k_ln_scale"], np.float32),
                        np.asarray(inputs["per_dim_scale"], np.float32))
    ctab = ctab.astype(bf16)
    masks = _host_masks().astype(bf16)

    in_maps = []
    for c in range(8):
        b, g = divmod(c, 4)
        hs = slice(NH * g, NH * (g + 1))
        wqkv_c = np.concatenate(
            [wq[:, hs, :].reshape(D, NH * HD),
             wk[:, hs, :].reshape(D, NH * HD),
             wv[:, hs, :].reshape(D, NH * HD)], axis=1)
        in_maps.append({
            "x": np.ascontiguousarray(x[b].T).astype(bf16),
            "wqkv": np.ascontiguousarray(wqkv_c).astype(bf16),
            "wo": np.ascontiguousarray(wo[hs].reshape(NH * HD, D)).astype(bf16),
            "ctab": ctab, "masks": masks,
            "ident": np.eye(128, dtype=bf16),
        })

    trace = os.environ.get("MHA_TRACE", "0") == "1"
    res = run_bass_kernel_spmd(nc, in_maps, list(range(8)), trace=trace)
    if trace:
        kernel.last_exec_time_ns = res.exec_time_ns
        kernel.last_results = res

    out = np.zeros((B, N, D), dtype=np.float32)
    for c in range(8):
        out[c // 4] += res.results[c]["outT"].T
    return out


# revision 24
# speedup vs baseline: 1.4435x; 1.0021x over previous
"""Trainium2 Bass kernel for nn_MultiHeadAttention_68152541053005.

Multi-head attention (B=2, N=2048, D=1024, H=16, d=64) with RoPE,
per-head RMSNorm on q/k, per-dim scale on q, causal softmax.

Sharding: 8 cores = 2 batch groups x 4 head-groups (4 heads/core).
Each core computes QKV projection for its 4 heads on its batch,
attention, and a partial output projection; the host sums the 4
partial outputs per batch (equivalent to the all-reduce after the
output projection).

Per-core kernel, software-pipelined across engines:
  - single interleaved instruction stream: QKV projection for token
    group g+1 is woven between attention matmuls for q-block g, so the
    PE never idles while ACT runs softmax exp (and stays at its max
    p-state, which needs ~3us of continuous PE work)
  - qT/kT produced by DMA XBAR transposes (no PE transposes)
  - ACT does exp + qk psum eviction only; DVE does psum-touching ops
    (v evict, reduce, rsqrt, recip, normalize, outproj evict); the
    Pool engine does sbuf-only elementwise (RoPE, squares, per-dim
    scale, causal masks)
  - softmax denominator rides as a ones-row in the ctx matmul; its
    reciprocal is partition-broadcast via a K=1 PE matmul
  - PSUM plan (8 banks): pqkv[128,768]f32 x2 bufs (4) + st[128,1024]
    f32 x1 (2) + ctx[128,512] x1 (1) + misc(po/bcast)[128,512] x1 (1)
"""

import os
import sys

if "/opt/trn_rl_repo" not in sys.path:
    sys.path.insert(0, "/opt/trn_rl_repo")

import numpy as np
from contextlib import ExitStack

import concourse.bacc as bacc
import concourse.bass as bass
import concourse.mybir as mybir
import concourse.tile as tile

AP = bass.AP
F32 = mybir.dt.float32
BF16 = mybir.dt.bfloat16
AFT = mybir.ActivationFunctionType
ALU = mybir.AluOpType

B, N, D, H, HD = 2, 2048, 1024, 16, 64
NH = 4            # heads per core
HALF = HD // 2    # 32
TC = N // 128     # 16 token chunks
DC = D // 128     # 8 D chunks
QB = N // 512     # 4 q blocks
LOG2_E = 1.442695041
RMS_EPS = 1e-6
MAX_TIMESCALE = 10000.0

# rsqrt(v) on DVE: z0 = c2*(v+h)^2 + k, then 2 Newton steps
# z <- z*(1.5 - 0.5*v*z^2); max rel err 8.5e-5 on v in [0.3, 2.3]
RS_H = -2.0157414099271302
RS_K = 0.6774616747941173
RS_C2 = 0.34740916
RS_VLO, RS_VHI = 0.3, 2.3


def _np_bf16():
    import ml_dtypes
    return np.dtype(ml_dtypes.bfloat16)


def build_nc():
    nc = bacc.Bacc("TRN2", target_bir_lowering=False, debug=False)

    x_d = nc.dram_tensor("x", [D, N], BF16, kind="ExternalInput")
    wqkv_d = nc.dram_tensor("wqkv", [D, 3 * NH * HD], BF16, kind="ExternalInput")
    wo_d = nc.dram_tensor("wo", [NH * HD, D], BF16, kind="ExternalInput")
    ctab_d = nc.dram_tensor("ctab", [N, 8 * HALF], BF16, kind="ExternalInput")
    masks_d = nc.dram_tensor("masks", [128, 128], BF16, kind="ExternalInput")
    ident_d = nc.dram_tensor("ident", [128, 128], BF16, kind="ExternalInput")
    outT_d = nc.dram_tensor("outT", [D, N], F32, kind="ExternalOutput")

    with tile.TileContext(nc) as tc, ExitStack() as ctx:
        build_tile_kernel(ctx, tc, x_d.ap(), wqkv_d.ap(), wo_d.ap(),
                          ctab_d.ap(), masks_d.ap(), ident_d.ap(),
                          outT_d.ap())
    nc.compile()
    return nc


def build_tile_kernel(ctx, tc, x, wqkv, wo, ctab, masks, identD, outT):
    nc = tc.nc

    res = ctx.enter_context(tc.tile_pool(name="res", bufs=1))
    stream = ctx.enter_context(tc.tile_pool(name="stream", bufs=3))
    scratch = ctx.enter_context(tc.tile_pool(name="scratch", bufs=2))
    qkpool = ctx.enter_context(tc.tile_pool(name="qkpool", bufs=3))
    ptp = ctx.enter_context(tc.tile_pool(name="ptp", bufs=3))
    ropep = ctx.enter_context(tc.tile_pool(name="ropep", bufs=5))

    drp = ctx.enter_context(tc.tile_pool(name="drp", bufs=2, space="DRAM"))
    pq = ctx.enter_context(tc.tile_pool(name="pq", bufs=1, space="PSUM"))
    pS = ctx.enter_context(tc.tile_pool(name="pS", bufs=2, space="PSUM"))
    pC = ctx.enter_context(tc.tile_pool(name="pC", bufs=1, space="PSUM"))
    pM = ctx.enter_context(tc.tile_pool(name="pM", bufs=1, space="PSUM"))

    # ---- resident constants ----
    wqkv_sb = res.tile([128, DC * 768], BF16, tag="wqkv")
    for c in range(DC):
        nc.sync.dma_start(wqkv_sb[:, 768 * c:768 * (c + 1)],
                          wqkv[128 * c:128 * (c + 1), :])
    wo_sb = res.tile([128, 2 * D], BF16, tag="wo")
    for r in range(2):
        nc.sync.dma_start(wo_sb[:, D * r:D * (r + 1)],
                          wo[128 * r:128 * (r + 1), :])
    masks_sb = res.tile([128, 128], BF16, tag="masks")
    nc.sync.dma_start(masks_sb[:], masks[:])
    ident = res.tile([128, 128], BF16, tag="ident")
    nc.sync.dma_start(ident[:], identD[:])

    qkT_all = res.tile([128, 4 * N], BF16, tag="qkT_all")
    qT = [qkT_all[:, i * N:(i + 1) * N] for i in range(2)]
    kT = [qkT_all[:, (2 + i) * N:(3 + i) * N] for i in range(2)]
    ctxT = [res.tile([128, N], BF16, tag=f"ctxT{i}", name=f"ctxT{i}")
            for i in range(2)]
    vt = res.tile([128, TC * NH * 65], BF16, tag="vt")
    va = vt[:]
    ones_dst = AP(va.tensor, va.offset + HD,
                  [va.ap[0], [NH * 65, TC], [65, NH], [1, 1]])
    nc.vector.memset(ones_dst, 1.0)

    # x is [D, N] in DRAM; stage the token-chunk columns in waves so
    # early chunks unblock quickly
    xT_big = [res.tile([128, N], BF16, tag=f"xT{c}", name=f"xTbig{c}")
              for c in range(DC)]
    for lo, hi in ((0, 128), (128, 512), (512, 1024), (1024, 2048)):
        for c in range(DC):
            nc.sync.dma_start(xT_big[c][:, lo:hi],
                              x[128 * c:128 * (c + 1), lo:hi])

    # ------------------------------------------------------------------
    # chunk pipeline: QKV projection + postproc for token chunk t
    # ------------------------------------------------------------------
    group_state = {}

    def make_chunk_units(t):
        state = {}
        gi, dt_i = divmod(t, 4)

        def mk(c):
            def u():
                if c == 0:
                    ctab_t = stream.tile([128, 256], BF16, tag="ctab",
                                         name=f"ctab{t}")
                    nc.sync.dma_start(ctab_t[:],
                                      ctab[128 * t:128 * (t + 1), :])
                    state["ctab"] = ctab_t
                    state["pqkv"] = pq.tile([128, 768], F32, tag="pqkv",
                                            name=f"pqkv{t}")
                pqkv = state["pqkv"]
                lhsT = xT_big[c][:, 128 * t:128 * (t + 1)]
                nc.tensor.matmul(pqkv[:, 0:512],
                                 lhsT, wqkv_sb[:, 768 * c:768 * c + 512],
                                 start=(c == 0), stop=(c == DC - 1))
                nc.tensor.matmul(pqkv[:, 512:768],
                                 lhsT, wqkv_sb[:, 768 * c + 512:768 * (c + 1)],
                                 start=(c == 0), stop=(c == DC - 1))
                if c == DC - 1:
                    post(state["pqkv"], state["ctab"])
            return (768, u)

        def post(pqkv, ctab_t):
            # group-final chunk: square straight from psum on ACT first,
            # so the stats chain starts before the eviction completes
            sq = scratch.tile([128, 512], BF16, tag="sq", name=f"sq{t}")
            if dt_i == 3:
                nc.scalar.square(sq[:], pqkv[:, 0:512])
            # qk eviction (ACT), v eviction (DVE, strided with ones gaps)
            qk_sb = qkpool.tile([128, 512], BF16, tag="qk_sb",
                                name=f"qk_sb{t}")
            nc.scalar.copy(qk_sb[:], pqkv[:, 0:512])
            va2 = vt[:]
            v_dst = AP(va2.tensor, va2.offset + NH * 65 * t,
                       [va2.ap[0], [65, NH], [1, HD]])
            pa = pqkv[:]
            v_src = AP(pa.tensor, pa.offset + 512,
                       [pa.ap[0], [HD, NH], [1, HD]])
            nc.vector.tensor_copy(v_dst, v_src)

            # RMSNorm stats from pre-RoPE q/k (rotation preserves norms);
            # reduce into the group's [128, 32] stats tile
            if dt_i != 3:
                nc.vector.tensor_mul(sq[:], qk_sb[:], qk_sb[:])
            if dt_i == 0:
                group_state[gi] = {
                    "ssq": scratch.tile([128, 32], F32, tag="ssq",
                                        name=f"ssq_g{gi}")
                }
            ssq = group_state[gi]["ssq"]
            nc.vector.reduce_sum(ssq[:, 8 * dt_i:8 * (dt_i + 1)],
                                 sq[:].rearrange("p (h d) -> p h d", d=HD),
                                 axis=mybir.AxisListType.X)

            # RoPE on DVE via host-folded tables (per-dim scales folded in)
            def dat(off, tl):
                a = tl[:]
                return AP(a.tensor, a.offset + off,
                          [a.ap[0], [256, 2], [HD, NH], [1, HALF]])

            def tab(f):
                a = ctab_t[:]
                return AP(a.tensor, a.offset + 64 * f,
                          [a.ap[0], [HALF, 2], [0, NH], [1, HALF]])

            tmp = [scratch.tile([128, 256], BF16, tag=f"rp{i}",
                                name=f"rp{i}_{t}") for i in range(4)]
            roped = ropep.tile([128, 512], BF16, tag="roped",
                               name=f"roped{t}")
            eng = nc.vector if dt_i == 3 else nc.gpsimd
            eng.tensor_mul(tmp[0][:], dat(0, qk_sb), tab(0))
            eng.tensor_mul(tmp[1][:], dat(HALF, qk_sb), tab(1))
            eng.tensor_sub(dat(0, roped), tmp[0][:], tmp[1][:])
            eng.tensor_mul(tmp[2][:], dat(HALF, qk_sb), tab(2))
            eng.tensor_mul(tmp[3][:], dat(0, qk_sb), tab(3))
            eng.tensor_add(dat(HALF, roped), tmp[2][:], tmp[3][:])
            group_state[gi][f"roped{dt_i}"] = roped

            if dt_i >= 2:
                # rsqrt (quadratic seed + 2 Newton steps); chunks 0-2
                # batch at dt 2 so their transposes aren't gated by
                # chunk 3; chunk 3 runs its own small chain
                lo, ncol = (0, 24) if dt_i == 2 else (24, 8)
                sl = slice(lo, lo + ncol)
                v_ = scratch.tile([128, 32], F32, tag="rsv",
                                  name=f"rsv{gi}_{dt_i}")
                nc.vector.tensor_scalar(v_[:, sl], ssq[:, sl], 1.0 / HD,
                                        RMS_EPS, ALU.mult, ALU.add)
                vc = scratch.tile([128, 32], F32, tag="rsvc",
                                  name=f"rsvc{gi}_{dt_i}")
                nc.vector.tensor_scalar(vc[:, sl], v_[:, sl], RS_VLO,
                                        RS_VHI, ALU.max, ALU.min)
                t_ = scratch.tile([128, 32], F32, tag="rst",
                                  name=f"rst{gi}_{dt_i}")
                nc.vector.tensor_scalar_add(t_[:, sl], vc[:, sl], RS_H)
                z_ = scratch.tile([128, 32], F32, tag="rsz",
                                  name=f"rsz{gi}_{dt_i}")
                nc.vector.scalar_tensor_tensor(z_[:, sl], t_[:, sl], RS_C2,
                                               t_[:, sl], ALU.mult, ALU.mult)
                nc.vector.tensor_scalar_add(z_[:, sl], z_[:, sl], RS_K)
                z2 = scratch.tile([128, 32], F32, tag="rsz2",
                                  name=f"rsz2{gi}_{dt_i}")
                w_ = scratch.tile([128, 32], F32, tag="rsw",
                                  name=f"rsw{gi}_{dt_i}")
                if dt_i == 2:
                    rs = scratch.tile([128, 32], F32, tag="rs_g",
                                      name=f"rs{gi}")
                    group_state[gi]["rs"] = rs
                else:
                    rs = group_state[gi]["rs"]
                for it in range(2):
                    nc.vector.tensor_mul(z2[:, sl], z_[:, sl], z_[:, sl])
                    nc.vector.scalar_tensor_tensor(w_[:, sl], z2[:, sl],
                                                   -0.5, v_[:, sl],
                                                   ALU.mult, ALU.mult)
                    out_ = rs if it == 1 else z_
                    out_sl = out_[:, sl] if it == 1 else out_[:, sl]
                    nc.vector.scalar_tensor_tensor(out_sl, w_[:, sl], 1.5,
                                                   z_[:, sl],
                                                   ALU.add, ALU.mult)

        return [mk(c) for c in range(DC)]

    def make_trans_unit(t):
        gi, dt_i = divmod(t, 4)

        def u():
            roped = group_state[gi].pop(f"roped{dt_i}")
            rs = group_state[gi]["rs"]
            # apply per-(token, head) rsqrt; broadcast over the 64 dims
            qk_stage = scratch.tile([128, 512], BF16, tag="qk_stage",
                                    name=f"qk_stage{t}")
            ra = rs[:]
            rs_b = AP(ra.tensor, ra.offset + 8 * dt_i,
                      [ra.ap[0], [1, 8], [0, HD]])
            nc.vector.tensor_mul(
                qk_stage[:].rearrange("p (h d) -> p h d", d=HD),
                roped[:].rearrange("p (h d) -> p h d", d=HD), rs_b)
            ptq = pM.tile([128, 512], BF16, tag="misc", name=f"ptq{t}")
            for i in range(4):
                nc.tensor.transpose(ptq[:, 128 * i:128 * (i + 1)],
                                    qk_stage[:, 128 * i:128 * (i + 1)],
                                    ident[:])
            qa_ = qkT_all[:]
            dst = AP(qa_.tensor, qa_.offset + 128 * t,
                     [qa_.ap[0], [N, 4], [1, 128]])
            nc.vector.tensor_copy(dst, ptq[:])
        return (512, u)

    # ------------------------------------------------------------------
    # attention for q block Q (512 queries), head h
    # ------------------------------------------------------------------
    def make_attn_units(Q):
        qcol = slice(512 * Q, 512 * (Q + 1))
        units = []
        q_state = {}
        for h in range(NH):
            g, off = divmod(h, 2)
            row = slice(64 * off, 64 * off + 64)
            npair = 2 * Q + 2
            st_state = {}

            def mk_st(p, g=g, row=row, h=h):
                def u():
                    pst = pS.tile([128, 1024], F32, tag="st",
                                  name=f"st{Q}_{h}_{p}")
                    regions = []
                    for s in range(2):
                        j = 2 * p + s
                        qoff = max(0, 128 * j - 512 * Q)
                        cols = 512 - qoff
                        nc.tensor.matmul(
                            pst[:, 512 * s:512 * s + cols],
                            kT[g][row, 128 * j:128 * (j + 1)],
                            qT[g][row, 512 * Q + qoff:512 * (Q + 1)],
                            start=True, stop=True)
                        regions.append(cols)
                    pt = ptp.tile([128, 1024], BF16, tag="pt",
                                  name=f"pt{Q}_{h}_{p}")
                    c0, c1 = regions
                    if c0 == 512:
                        nc.scalar.activation(pt[:, 0:512 + c1],
                                             pst[:, 0:512 + c1], AFT.Exp)
                    else:
                        nc.scalar.activation(pt[:, 0:c0], pst[:, 0:c0],
                                             AFT.Exp)
                        nc.scalar.activation(pt[:, 512:512 + c1],
                                             pst[:, 512:512 + c1], AFT.Exp)
                    if p >= 2 * Q:
                        nc.vector.tensor_mul(pt[:, 0:128], pt[:, 0:128],
                                             masks_sb[:])
                        nc.vector.tensor_mul(pt[:, 512:640], pt[:, 512:640],
                                             masks_sb[:])
                    st_state[p] = pt
                cost = sum(512 - max(0, 128 * (2 * p + s) - 512 * Q)
                           for s in range(2))
                return (cost, u)

            def mk_ctx(p, h=h, npair=npair):
                def u():
                    if p == 0:
                        st_state["pctx"] = pC.tile([65, 512], F32, tag="ctx",
                                                   name=f"ctx{Q}_{h}")
                    pctx = st_state["pctx"]
                    pt = st_state.pop(p)
                    for s in range(2):
                        j = 2 * p + s
                        qoff = max(0, 128 * j - 512 * Q)
                        cols = 512 - qoff
                        nc.tensor.matmul(
                            pctx[:, qoff:512],
                            vt[:, 65 * (NH * j + h):65 * (NH * j + h) + 65],
                            pt[:, 512 * s:512 * s + cols],
                            start=(j == 0), stop=(j == 2 * npair - 1))
                cost = sum(512 - max(0, 128 * (2 * p + s) - 512 * Q)
                           for s in range(2))
                return (cost, u)

            def mk_evict(h=h):
                def u():
                    pctx = st_state.pop("pctx")
                    # evict raw ctx + den to SBUF fast to free the psum
                    # bank; the actual normalize is batched per q block
                    if h == 0:
                        q_state["u"] = scratch.tile(
                            [64, 4 * 512], F32, tag="u_sb",
                            name=f"u{Q}", bufs=2)
                        q_state["den"] = scratch.tile(
                            [1, 4 * 512], F32, tag="den_sb",
                            name=f"den{Q}", bufs=2)
                    nc.vector.tensor_copy(
                        q_state["u"][:, 512 * h:512 * (h + 1)], pctx[0:64, :])
                    nc.vector.tensor_copy(
                        q_state["den"][:, 512 * h:512 * (h + 1)],
                        pctx[64:65, :])
                return (0, u)

            units.append(mk_st(0))
            for p in range(1, npair):
                units.append(mk_st(p))
                units.append(mk_ctx(p - 1))
            units.append(mk_ctx(npair - 1))
            units.append(mk_evict())

        def mk_norm_all():
            def u():
                u_sb, den_sb = q_state.pop("u"), q_state.pop("den")
                recip0 = scratch.tile([1, 4 * 512], F32, tag="recip0",
                                      name=f"recip{Q}")
                nc.vector.reciprocal_approx_fast(recip0[:], den_sb[:])
                bcast = scratch.tile([64, 4 * 512], F32, tag="bcast",
                                     name=f"bcast{Q}")
                rb = drp.tile([1, 4 * 512], F32, tag="rb", name=f"rb{Q}")
                nc.sync.dma_start(rb[:], recip0[:])
                ra = rb[:]
                src_b = AP(ra.tensor, ra.offset, [[0, 64], [1, 4 * 512]])
                nc.sync.dma_start(bcast[:], src_b)
                for h in range(NH):
                    g, off = divmod(h, 2)
                    row = slice(64 * off, 64 * off + 64)
                    nc.vector.tensor_mul(ctxT[g][row, qcol],
                                         u_sb[:, 512 * h:512 * (h + 1)],
                                         bcast[:, 512 * h:512 * (h + 1)])
            return (0, u)

        units.append(mk_norm_all())
        return units

    # ------------------------------------------------------------------
    # output projection for q block Q
    # ------------------------------------------------------------------
    def make_outproj_units(Q):
        qcol = slice(512 * Q, 512 * (Q + 1))
        units = []
        for m in range(DC):
            def u(m=m):
                po = pM.tile([128, 512], F32, tag="misc",
                             name=f"po{Q}_{m}")
                for r in range(2):
                    nc.tensor.matmul(
                        po[:],
                        wo_sb[:, D * r + 128 * m:D * r + 128 * (m + 1)],
                        ctxT[r][:, qcol], start=(r == 0), stop=(r == 1))
                ob = scratch.tile([128, 512], F32, tag="ob",
                                  name=f"ob{Q}_{m}")
                nc.vector.tensor_copy(ob[:], po[:])
                nc.sync.dma_start(outT[128 * m:128 * (m + 1), qcol], ob[:])
            units.append((1024, u))
        return units

    # ------------------------------------------------------------------
    # weave two unit streams proportionally by PE cost
    # ------------------------------------------------------------------
    def weave(A, B):
        totA = sum(c for c, _ in A) or 1
        totB = sum(c for c, _ in B) or 1
        out = []
        ia = ib = 0
        ca = cb = 0
        while ia < len(A) or ib < len(B):
            fa = ca / totA
            fb = cb / totB
            if ib >= len(B) or (ia < len(A) and fa <= fb):
                c, u = A[ia]; ia += 1; ca += c
            else:
                c, u = B[ib]; ib += 1; cb += c
            out.append(u)
        return out

    def make_group_units(g):
        units = []
        for t in range(4 * g, 4 * (g + 1)):
            units += make_chunk_units(t)
        for t in range(4 * g, 4 * (g + 1)):
            units.append(make_trans_unit(t))
        return units

    def weave_costed(A, B):
        totA = sum(c for c, _ in A) or 1
        totB = sum(c for c, _ in B) or 1
        out = []
        ia = ib = 0
        ca = cb = 0
        while ia < len(A) or ib < len(B):
            fa = ca / totA
            fb = cb / totB
            if ib >= len(B) or (ia < len(A) and fa <= fb):
                c, u = A[ia]; ia += 1; ca += c
            else:
                c, u = B[ib]; ib += 1; cb += c
            out.append((c, u))
        return out

    plan = []
    # group 0 alone (attention needs its k/v first)
    plan += [u for _, u in make_group_units(0)]
    # attn(Q) ∥ chunks of group Q+1 ∥ outproj(Q-1)
    for Q in range(3):
        other = make_attn_units(Q)
        if Q:
            other = weave_costed(other, make_outproj_units(Q - 1))
        plan += weave(other, make_group_units(Q + 1))
    plan += weave(weave_costed(make_attn_units(3), make_outproj_units(2)),
                  [])
    plan += [u for _, u in make_outproj_units(3)]

    for u in plan:
        u()


# ---------------------------------------------------------------------------
# host side
# ---------------------------------------------------------------------------

_CACHE = {}


def _get_nc():
    if "nc" not in _CACHE:
        _CACHE["nc"] = build_nc()
    return _CACHE["nc"]


def _host_tables(q_ln_scale, k_ln_scale, per_dim_scale):
    frac = 2.0 * np.arange(HALF, dtype=np.float32) / HD
    ts = (MAX_TIMESCALE ** frac).astype(np.float32)
    pos = np.arange(N, dtype=np.float32)
    sinu = pos[:, None] / ts[None, :]
    SIN = np.sin(sinu).astype(np.float32)
    COS = np.cos(sinu).astype(np.float32)
    qs = (LOG2_E / np.sqrt(np.float32(HD))
          * np.logaddexp(0.0, per_dim_scale.astype(np.float64))).astype(np.float32)
    qscale = (q_ln_scale * qs).astype(np.float32)
    kscale = k_ln_scale.astype(np.float32)

    # combined table [N, 256]: func f in {cosA,sinA,cosB,sinB} at cols
    # [64f:64f+64], q-scaled half at +0:32, k-scaled at +32:64
    blocks = []
    for base, half in ((COS, slice(0, HALF)), (SIN, slice(0, HALF)),
                       (COS, slice(HALF, HD)), (SIN, slice(HALF, HD))):
        blocks.append(base * qscale[None, half])
        blocks.append(base * kscale[None, half])
    return np.concatenate(blocks, axis=1).astype(np.float32)


def _host_masks():
    # mask[r, c] = 1 if c >= r  (S.T block: rows k, cols q)
    r = np.arange(128)[:, None]
    c = np.arange(128)[None, :]
    return (c >= r).astype(np.float32)


def kernel(**inputs):
    from concourse.bass_utils import run_bass_kernel_spmd

    nc = _get_nc()
    bf16 = _np_bf16()

    x = np.asarray(inputs["inputs_q"], dtype=np.float32)
    wq = np.asarray(inputs["wq"], dtype=np.float32)
    wk = np.asarray(inputs["wk"], dtype=np.float32)
    wv = np.asarray(inputs["wv"], dtype=np.float32)
    wo = np.asarray(inputs["wo"], dtype=np.float32)

    ctab = _host_tables(np.asarray(inputs["q_ln_scale"], np.float32),
                        np.asarray(inputs["k_ln_scale"], np.float32),
                        np.asarray(inputs["per_dim_scale"], np.float32))
    ctab = ctab.astype(bf16)
    masks = _host_masks().astype(bf16)

    in_maps = []
    for c in range(8):
        b, g = divmod(c, 4)
        hs = slice(NH * g, NH * (g + 1))
        wqkv_c = np.concatenate(
            [wq[:, hs, :].reshape(D, NH * HD),
             wk[:, hs, :].reshape(D, NH * HD),
             wv[:, hs, :].reshape(D, NH * HD)], axis=1)
        in_maps.append({
            "x": np.ascontiguousarray(x[b].T).astype(bf16),
            "wqkv": np.ascontiguousarray(wqkv_c).astype(bf16),
            "wo": np.ascontiguousarray(wo[hs].reshape(NH * HD, D)).astype(bf16),
            "ctab": ctab, "masks": masks,
            "ident": np.eye(128, dtype=bf16),
        })

    trace = os.environ.get("MHA_TRACE", "0") == "1"
    res = run_bass_kernel_spmd(nc, in_maps, list(range(8)), trace=trace)
    if trace:
        kernel.last_exec_time_ns = res.exec_time_ns
        kernel.last_results = res

    out = np.zeros((B, N, D), dtype=np.float32)
    for c in range(8):
        out[c // 4] += res.results[c]["outT"].T
    return out


# revision 25
# speedup vs baseline: 1.4697x; 1.0182x over previous
"""Trainium2 Bass kernel for nn_MultiHeadAttention_68152541053005.

Multi-head attention (B=2, N=2048, D=1024, H=16, d=64) with RoPE,
per-head RMSNorm on q/k, per-dim scale on q, causal softmax.

Sharding: 8 cores = 2 batch groups x 4 head-groups (4 heads/core).
Each core computes QKV projection for its 4 heads on its batch,
attention, and a partial output projection; the host sums the 4
partial outputs per batch (equivalent to the all-reduce after the
output projection).

Per-core kernel, software-pipelined across engines:
  - single interleaved instruction stream: QKV projection for token
    group g+1 is woven between attention matmuls for q-block g, so the
    PE never idles while ACT runs softmax exp (and stays at its max
    p-state, which needs ~3us of continuous PE work)
  - qT/kT produced by DMA XBAR transposes (no PE transposes)
  - ACT does exp + qk psum eviction only; DVE does psum-touching ops
    (v evict, reduce, rsqrt, recip, normalize, outproj evict); the
    Pool engine does sbuf-only elementwise (RoPE, squares, per-dim
    scale, causal masks)
  - softmax denominator rides as a ones-row in the ctx matmul; its
    reciprocal is partition-broadcast via a K=1 PE matmul
  - PSUM plan (8 banks): pqkv[128,768]f32 x2 bufs (4) + st[128,1024]
    f32 x1 (2) + ctx[128,512] x1 (1) + misc(po/bcast)[128,512] x1 (1)
"""

import os
import sys

if "/opt/trn_rl_repo" not in sys.path:
    sys.path.insert(0, "/opt/trn_rl_repo")

import numpy as np
from contextlib import ExitStack

import concourse.bacc as bacc
import concourse.bass as bass
import concourse.mybir as mybir
import concourse.tile as tile

AP = bass.AP
F32 = mybir.dt.float32
BF16 = mybir.dt.bfloat16
AFT = mybir.ActivationFunctionType
ALU = mybir.AluOpType

B, N, D, H, HD = 2, 2048, 1024, 16, 64
NH = 4            # heads per core
HALF = HD // 2    # 32
TC = N // 128     # 16 token chunks
DC = D // 128     # 8 D chunks
QB = N // 512     # 4 q blocks
LOG2_E = 1.442695041
RMS_EPS = 1e-6
MAX_TIMESCALE = 10000.0

# rsqrt(v) on DVE: z0 = c2*(v+h)^2 + k, then 2 Newton steps
# z <- z*(1.5 - 0.5*v*z^2); max rel err 8.5e-5 on v in [0.3, 2.3]
RS_H = -2.0157414099271302
RS_K = 0.6774616747941173
RS_C2 = 0.34740916
RS_VLO, RS_VHI = 0.3, 2.3


def _np_bf16():
    import ml_dtypes
    return np.dtype(ml_dtypes.bfloat16)


def build_nc():
    nc = bacc.Bacc("TRN2", target_bir_lowering=False, debug=False)

    x_d = nc.dram_tensor("x", [D, N], BF16, kind="ExternalInput")
    wqkv_d = nc.dram_tensor("wqkv", [D, 3 * NH * HD], BF16, kind="ExternalInput")
    wo_d = nc.dram_tensor("wo", [NH * HD, D], BF16, kind="ExternalInput")
    ctab_d = nc.dram_tensor("ctab", [N, 8 * HALF], BF16, kind="ExternalInput")
    masks_d = nc.dram_tensor("masks", [128, 128], BF16, kind="ExternalInput")
    ident_d = nc.dram_tensor("ident", [128, 128], BF16, kind="ExternalInput")
    outT_d = nc.dram_tensor("outT", [D, N], F32, kind="ExternalOutput")

    with tile.TileContext(nc) as tc, ExitStack() as ctx:
        build_tile_kernel(ctx, tc, x_d.ap(), wqkv_d.ap(), wo_d.ap(),
                          ctab_d.ap(), masks_d.ap(), ident_d.ap(),
                          outT_d.ap())
    nc.compile()
    return nc


def build_tile_kernel(ctx, tc, x, wqkv, wo, ctab, masks, identD, outT):
    nc = tc.nc

    res = ctx.enter_context(tc.tile_pool(name="res", bufs=1))
    stream = ctx.enter_context(tc.tile_pool(name="stream", bufs=3))
    scratch = ctx.enter_context(tc.tile_pool(name="scratch", bufs=2))
    qkpool = ctx.enter_context(tc.tile_pool(name="qkpool", bufs=3))
    ptp = ctx.enter_context(tc.tile_pool(name="ptp", bufs=3))
    ropep = ctx.enter_context(tc.tile_pool(name="ropep", bufs=5))

    drp = ctx.enter_context(tc.tile_pool(name="drp", bufs=2, space="DRAM"))
    pq = ctx.enter_context(tc.tile_pool(name="pq", bufs=1, space="PSUM"))
    pS = ctx.enter_context(tc.tile_pool(name="pS", bufs=2, space="PSUM"))
    pC = ctx.enter_context(tc.tile_pool(name="pC", bufs=1, space="PSUM"))
    pM = ctx.enter_context(tc.tile_pool(name="pM", bufs=1, space="PSUM"))

    # ---- resident constants ----
    masks_sb = res.tile([128, 128], BF16, tag="masks")
    nc.sync.dma_start(masks_sb[:], masks[:])
    ident = res.tile([128, 128], BF16, tag="ident")
    nc.sync.dma_start(ident[:], identD[:])

    qkT_all = res.tile([128, 4 * N], BF16, tag="qkT_all")
    qT = [qkT_all[:, i * N:(i + 1) * N] for i in range(2)]
    kT = [qkT_all[:, (2 + i) * N:(3 + i) * N] for i in range(2)]
    ctxT = [res.tile([128, N], BF16, tag=f"ctxT{i}", name=f"ctxT{i}")
            for i in range(2)]
    vt = res.tile([128, TC * NH * 65], BF16, tag="vt")
    va = vt[:]
    ones_dst = AP(va.tensor, va.offset + HD,
                  [va.ap[0], [NH * 65, TC], [65, NH], [1, 1]])
    nc.vector.memset(ones_dst, 1.0)

    # x is [D, N] in DRAM; stage the token-chunk columns in waves so
    # early chunks unblock quickly
    xT_big = [res.tile([128, N], BF16, tag=f"xT{c}", name=f"xTbig{c}")
              for c in range(DC)]
    wqkv_sb = res.tile([128, DC * 768], BF16, tag="wqkv")
    wo_sb = res.tile([128, 2 * D], BF16, tag="wo")
    for c in range(DC):
        nc.sync.dma_start(xT_big[c][:, 0:128], x[128 * c:128 * (c + 1), 0:128])
    for c in range(DC):
        nc.sync.dma_start(wqkv_sb[:, 768 * c:768 * (c + 1)],
                          wqkv[128 * c:128 * (c + 1), :])
    for lo, hi in ((128, 512), (512, 1024), (1024, 2048)):
        for c in range(DC):
            nc.sync.dma_start(xT_big[c][:, lo:hi],
                              x[128 * c:128 * (c + 1), lo:hi])
    for r in range(2):
        nc.sync.dma_start(wo_sb[:, D * r:D * (r + 1)],
                          wo[128 * r:128 * (r + 1), :])

    # ------------------------------------------------------------------
    # chunk pipeline: QKV projection + postproc for token chunk t
    # ------------------------------------------------------------------
    group_state = {}

    def make_chunk_units(t):
        state = {}
        gi, dt_i = divmod(t, 4)

        def mk(c):
            def u():
                if c == 0:
                    ctab_t = stream.tile([128, 256], BF16, tag="ctab",
                                         name=f"ctab{t}")
                    nc.sync.dma_start(ctab_t[:],
                                      ctab[128 * t:128 * (t + 1), :])
                    state["ctab"] = ctab_t
                    state["pqkv"] = pq.tile([128, 768], F32, tag="pqkv",
                                            name=f"pqkv{t}")
                pqkv = state["pqkv"]
                lhsT = xT_big[c][:, 128 * t:128 * (t + 1)]
                nc.tensor.matmul(pqkv[:, 0:512],
                                 lhsT, wqkv_sb[:, 768 * c:768 * c + 512],
                                 start=(c == 0), stop=(c == DC - 1))
                nc.tensor.matmul(pqkv[:, 512:768],
                                 lhsT, wqkv_sb[:, 768 * c + 512:768 * (c + 1)],
                                 start=(c == 0), stop=(c == DC - 1))
                if c == DC - 1:
                    post(state["pqkv"], state["ctab"])
            return (768, u)

        def post(pqkv, ctab_t):
            # group-final chunk: square straight from psum on ACT first,
            # so the stats chain starts before the eviction completes
            sq = scratch.tile([128, 512], BF16, tag="sq", name=f"sq{t}")
            if dt_i == 3:
                nc.scalar.square(sq[:], pqkv[:, 0:512])
            # qk eviction (ACT), v eviction (DVE, strided with ones gaps)
            qk_sb = qkpool.tile([128, 512], BF16, tag="qk_sb",
                                name=f"qk_sb{t}")
            nc.scalar.copy(qk_sb[:], pqkv[:, 0:512])
            va2 = vt[:]
            v_dst = AP(va2.tensor, va2.offset + NH * 65 * t,
                       [va2.ap[0], [65, NH], [1, HD]])
            pa = pqkv[:]
            v_src = AP(pa.tensor, pa.offset + 512,
                       [pa.ap[0], [HD, NH], [1, HD]])
            nc.vector.tensor_copy(v_dst, v_src)

            # RMSNorm stats from pre-RoPE q/k (rotation preserves norms);
            # reduce into the group's [128, 32] stats tile
            if dt_i != 3:
                nc.vector.tensor_mul(sq[:], qk_sb[:], qk_sb[:])
            if dt_i == 0:
                group_state[gi] = {
                    "ssq": scratch.tile([128, 32], F32, tag="ssq",
                                        name=f"ssq_g{gi}")
                }
            ssq = group_state[gi]["ssq"]
            nc.vector.reduce_sum(ssq[:, 8 * dt_i:8 * (dt_i + 1)],
                                 sq[:].rearrange("p (h d) -> p h d", d=HD),
                                 axis=mybir.AxisListType.X)

            # RoPE on DVE via host-folded tables (per-dim scales folded in)
            def dat(off, tl):
                a = tl[:]
                return AP(a.tensor, a.offset + off,
                          [a.ap[0], [256, 2], [HD, NH], [1, HALF]])

            def tab(f):
                a = ctab_t[:]
                return AP(a.tensor, a.offset + 64 * f,
                          [a.ap[0], [HALF, 2], [0, NH], [1, HALF]])

            tmp = [scratch.tile([128, 256], BF16, tag=f"rp{i}",
                                name=f"rp{i}_{t}") for i in range(4)]
            roped = ropep.tile([128, 512], BF16, tag="roped",
                               name=f"roped{t}")
            eng = nc.vector if dt_i == 3 else nc.gpsimd
            eng.tensor_mul(tmp[0][:], dat(0, qk_sb), tab(0))
            eng.tensor_mul(tmp[1][:], dat(HALF, qk_sb), tab(1))
            eng.tensor_sub(dat(0, roped), tmp[0][:], tmp[1][:])
            eng.tensor_mul(tmp[2][:], dat(HALF, qk_sb), tab(2))
            eng.tensor_mul(tmp[3][:], dat(0, qk_sb), tab(3))
            eng.tensor_add(dat(HALF, roped), tmp[2][:], tmp[3][:])
            group_state[gi][f"roped{dt_i}"] = roped

            if dt_i >= 2:
                # rsqrt (quadratic seed + 2 Newton steps); chunks 0-2
                # batch at dt 2 so their transposes aren't gated by
                # chunk 3; chunk 3 runs its own small chain
                lo, ncol = (0, 24) if dt_i == 2 else (24, 8)
                sl = slice(lo, lo + ncol)
                v_ = scratch.tile([128, 32], F32, tag="rsv",
                                  name=f"rsv{gi}_{dt_i}")
                nc.vector.tensor_scalar(v_[:, sl], ssq[:, sl], 1.0 / HD,
                                        RMS_EPS, ALU.mult, ALU.add)
                vc = scratch.tile([128, 32], F32, tag="rsvc",
                                  name=f"rsvc{gi}_{dt_i}")
                nc.vector.tensor_scalar(vc[:, sl], v_[:, sl], RS_VLO,
                                        RS_VHI, ALU.max, ALU.min)
                t_ = scratch.tile([128, 32], F32, tag="rst",
                                  name=f"rst{gi}_{dt_i}")
                nc.vector.tensor_scalar_add(t_[:, sl], vc[:, sl], RS_H)
                z_ = scratch.tile([128, 32], F32, tag="rsz",
                                  name=f"rsz{gi}_{dt_i}")
                nc.vector.scalar_tensor_tensor(z_[:, sl], t_[:, sl], RS_C2,
                                               t_[:, sl], ALU.mult, ALU.mult)
                nc.vector.tensor_scalar_add(z_[:, sl], z_[:, sl], RS_K)
                z2 = scratch.tile([128, 32], F32, tag="rsz2",
                                  name=f"rsz2{gi}_{dt_i}")
                w_ = scratch.tile([128, 32], F32, tag="rsw",
                                  name=f"rsw{gi}_{dt_i}")
                if dt_i == 2:
                    rs = scratch.tile([128, 32], F32, tag="rs_g",
                                      name=f"rs{gi}")
                    group_state[gi]["rs"] = rs
                else:
                    rs = group_state[gi]["rs"]
                for it in range(2):
                    nc.vector.tensor_mul(z2[:, sl], z_[:, sl], z_[:, sl])
                    nc.vector.scalar_tensor_tensor(w_[:, sl], z2[:, sl],
                                                   -0.5, v_[:, sl],
                                                   ALU.mult, ALU.mult)
                    out_ = rs if it == 1 else z_
                    out_sl = out_[:, sl] if it == 1 else out_[:, sl]
                    nc.vector.scalar_tensor_tensor(out_sl, w_[:, sl], 1.5,
                                                   z_[:, sl],
                                                   ALU.add, ALU.mult)

        return [mk(c) for c in range(DC)]

    def make_trans_unit(t):
        gi, dt_i = divmod(t, 4)

        def u():
            roped = group_state[gi].pop(f"roped{dt_i}")
            rs = group_state[gi]["rs"]
            # apply per-(token, head) rsqrt; broadcast over the 64 dims
            qk_stage = scratch.tile([128, 512], BF16, tag="qk_stage",
                                    name=f"qk_stage{t}")
            ra = rs[:]
            rs_b = AP(ra.tensor, ra.offset + 8 * dt_i,
                      [ra.ap[0], [1, 8], [0, HD]])
            nc.vector.tensor_mul(
                qk_stage[:].rearrange("p (h d) -> p h d", d=HD),
                roped[:].rearrange("p (h d) -> p h d", d=HD), rs_b)
            ptq = pM.tile([128, 512], BF16, tag="misc", name=f"ptq{t}")
            for i in range(4):
                nc.tensor.transpose(ptq[:, 128 * i:128 * (i + 1)],
                                    qk_stage[:, 128 * i:128 * (i + 1)],
                                    ident[:])
            qa_ = qkT_all[:]
            dst = AP(qa_.tensor, qa_.offset + 128 * t,
                     [qa_.ap[0], [N, 4], [1, 128]])
            nc.vector.tensor_copy(dst, ptq[:])
        return (512, u)

    # ------------------------------------------------------------------
    # attention for q block Q (512 queries), head h
    # ------------------------------------------------------------------
    def make_attn_units(Q):
        qcol = slice(512 * Q, 512 * (Q + 1))
        units = []
        q_state = {}
        for h in range(NH):
            g, off = divmod(h, 2)
            row = slice(64 * off, 64 * off + 64)
            npair = 2 * Q + 2
            st_state = {}

            def mk_st(p, g=g, row=row, h=h):
                def u():
                    pst = pS.tile([128, 1024], F32, tag="st",
                                  name=f"st{Q}_{h}_{p}")
                    regions = []
                    for s in range(2):
                        j = 2 * p + s
                        qoff = max(0, 128 * j - 512 * Q)
                        cols = 512 - qoff
                        nc.tensor.matmul(
                            pst[:, 512 * s:512 * s + cols],
                            kT[g][row, 128 * j:128 * (j + 1)],
                            qT[g][row, 512 * Q + qoff:512 * (Q + 1)],
                            start=True, stop=True)
                        regions.append(cols)
                    pt = ptp.tile([128, 1024], BF16, tag="pt",
                                  name=f"pt{Q}_{h}_{p}")
                    c0, c1 = regions
                    if c0 == 512:
                        nc.scalar.activation(pt[:, 0:512 + c1],
                                             pst[:, 0:512 + c1], AFT.Exp)
                    else:
                        nc.scalar.activation(pt[:, 0:c0], pst[:, 0:c0],
                                             AFT.Exp)
                        nc.scalar.activation(pt[:, 512:512 + c1],
                                             pst[:, 512:512 + c1], AFT.Exp)
                    if p >= 2 * Q:
                        nc.vector.tensor_mul(pt[:, 0:128], pt[:, 0:128],
                                             masks_sb[:])
                        nc.vector.tensor_mul(pt[:, 512:640], pt[:, 512:640],
                                             masks_sb[:])
                    st_state[p] = pt
                cost = sum(512 - max(0, 128 * (2 * p + s) - 512 * Q)
                           for s in range(2))
                return (cost, u)

            def mk_ctx(p, h=h, npair=npair):
                def u():
                    if p == 0:
                        st_state["pctx"] = pC.tile([65, 512], F32, tag="ctx",
                                                   name=f"ctx{Q}_{h}")
                    pctx = st_state["pctx"]
                    pt = st_state.pop(p)
                    for s in range(2):
                        j = 2 * p + s
                        qoff = max(0, 128 * j - 512 * Q)
                        cols = 512 - qoff
                        nc.tensor.matmul(
                            pctx[:, qoff:512],
                            vt[:, 65 * (NH * j + h):65 * (NH * j + h) + 65],
                            pt[:, 512 * s:512 * s + cols],
                            start=(j == 0), stop=(j == 2 * npair - 1))
                cost = sum(512 - max(0, 128 * (2 * p + s) - 512 * Q)
                           for s in range(2))
                return (cost, u)

            def mk_evict(h=h, g=g, row=row):
                def u():
                    pctx = st_state.pop("pctx")
                    if Q == QB - 1:
                        # final q block: normalize per head immediately
                        # (no chunk work left to hide a batched tail)
                        den1 = scratch.tile([1, 512], F32, tag="den1",
                                            name=f"den1_{Q}_{h}")
                        nc.vector.tensor_copy(den1[:], pctx[64:65, :])
                        rc1 = scratch.tile([1, 512], F32, tag="rc1",
                                           name=f"rc1_{Q}_{h}")
                        nc.vector.reciprocal_approx_fast(rc1[:], den1[:])
                        bc1 = scratch.tile([64, 512], F32, tag="bc1",
                                           name=f"bc1_{Q}_{h}")
                        nc.gpsimd.partition_broadcast(bc1[:], rc1[:])
                        nc.vector.tensor_mul(ctxT[g][row, qcol],
                                             pctx[0:64, :], bc1[:])
                        return
                    # evict raw ctx + den to SBUF fast to free the psum
                    # bank; the actual normalize is batched per q block
                    if h == 0:
                        q_state["u"] = scratch.tile(
                            [64, 4 * 512], F32, tag="u_sb",
                            name=f"u{Q}", bufs=2)
                        q_state["den"] = scratch.tile(
                            [1, 4 * 512], F32, tag="den_sb",
                            name=f"den{Q}", bufs=2)
                    nc.vector.tensor_copy(
                        q_state["u"][:, 512 * h:512 * (h + 1)], pctx[0:64, :])
                    nc.vector.tensor_copy(
                        q_state["den"][:, 512 * h:512 * (h + 1)],
                        pctx[64:65, :])
                return (0, u)

            units.append(mk_st(0))
            for p in range(1, npair):
                units.append(mk_st(p))
                units.append(mk_ctx(p - 1))
            units.append(mk_ctx(npair - 1))
            units.append(mk_evict())

        def mk_norm_all():
            def u():
                u_sb, den_sb = q_state.pop("u"), q_state.pop("den")
                recip0 = scratch.tile([1, 4 * 512], F32, tag="recip0",
                                      name=f"recip{Q}")
                nc.vector.reciprocal_approx_fast(recip0[:], den_sb[:])
                bcast = scratch.tile([64, 4 * 512], F32, tag="bcast",
                                     name=f"bcast{Q}")
                rb = drp.tile([1, 4 * 512], F32, tag="rb", name=f"rb{Q}")
                nc.sync.dma_start(rb[:], recip0[:])
                ra = rb[:]
                src_b = AP(ra.tensor, ra.offset, [[0, 64], [1, 4 * 512]])
                nc.sync.dma_start(bcast[:], src_b)
                for h in range(NH):
                    g, off = divmod(h, 2)
                    row = slice(64 * off, 64 * off + 64)
                    nc.vector.tensor_mul(ctxT[g][row, qcol],
                                         u_sb[:, 512 * h:512 * (h + 1)],
                                         bcast[:, 512 * h:512 * (h + 1)])
            return (0, u)

        if Q < QB - 1:
            units.append(mk_norm_all())
        return units

    # ------------------------------------------------------------------
    # output projection for q block Q
    # ------------------------------------------------------------------
    def make_outproj_units(Q):
        qcol = slice(512 * Q, 512 * (Q + 1))
        units = []
        for m in range(DC):
            def u(m=m):
                po = pM.tile([128, 512], F32, tag="misc",
                             name=f"po{Q}_{m}")
                for r in range(2):
                    nc.tensor.matmul(
                        po[:],
                        wo_sb[:, D * r + 128 * m:D * r + 128 * (m + 1)],
                        ctxT[r][:, qcol], start=(r == 0), stop=(r == 1))
                ob = scratch.tile([128, 512], F32, tag="ob",
                                  name=f"ob{Q}_{m}")
                nc.vector.tensor_copy(ob[:], po[:])
                nc.sync.dma_start(outT[128 * m:128 * (m + 1), qcol], ob[:])
            units.append((1024, u))
        return units

    # ------------------------------------------------------------------
    # weave two unit streams proportionally by PE cost
    # ------------------------------------------------------------------
    def weave(A, B):
        totA = sum(c for c, _ in A) or 1
        totB = sum(c for c, _ in B) or 1
        out = []
        ia = ib = 0
        ca = cb = 0
        while ia < len(A) or ib < len(B):
            fa = ca / totA
            fb = cb / totB
            if ib >= len(B) or (ia < len(A) and fa <= fb):
                c, u = A[ia]; ia += 1; ca += c
            else:
                c, u = B[ib]; ib += 1; cb += c
            out.append(u)
        return out

    def make_group_units(g):
        t0 = 4 * g
        units = []
        for t in range(t0, t0 + 3):
            units += make_chunk_units(t)
        c3 = make_chunk_units(t0 + 3)
        # chunks 0-2 transposes only need the dt2-batch rsqrt: weave
        # them between chunk 3's matmuls so the PE never drains
        tr = [make_trans_unit(t) for t in range(t0, t0 + 3)]
        units += [c3[0], c3[1], c3[2], tr[0], c3[3], c3[4], tr[1],
                  c3[5], c3[6], tr[2], c3[7]]
        units.append(make_trans_unit(t0 + 3))
        return units

    def weave_costed(A, B):
        totA = sum(c for c, _ in A) or 1
        totB = sum(c for c, _ in B) or 1
        out = []
        ia = ib = 0
        ca = cb = 0
        while ia < len(A) or ib < len(B):
            fa = ca / totA
            fb = cb / totB
            if ib >= len(B) or (ia < len(A) and fa <= fb):
                c, u = A[ia]; ia += 1; ca += c
            else:
                c, u = B[ib]; ib += 1; cb += c
            out.append((c, u))
        return out

    plan = []
    # group 0 alone (attention needs its k/v first)
    plan += [u for _, u in make_group_units(0)]
    # attn(Q) ∥ chunks of group Q+1 ∥ outproj(Q-1)
    for Q in range(3):
        other = make_attn_units(Q)
        if Q:
            other = weave_costed(other, make_outproj_units(Q - 1))
        plan += weave(other, make_group_units(Q + 1))
    plan += weave(weave_costed(make_attn_units(3), make_outproj_units(2)),
                  [])
    plan += [u for _, u in make_outproj_units(3)]

    for u in plan:
        u()


# ---------------------------------------------------------------------------
# host side
# ---------------------------------------------------------------------------

_CACHE = {}


def _get_nc():
    if "nc" not in _CACHE:
        _CACHE["nc"] = build_nc()
    return _CACHE["nc"]


def _host_tables(q_ln_scale, k_ln_scale, per_dim_scale):
    frac = 2.0 * np.arange(HALF, dtype=np.float32) / HD
    ts = (MAX_TIMESCALE ** frac).astype(np.float32)
    pos = np.arange(N, dtype=np.float32)
    sinu = pos[:, None] / ts[None, :]
    SIN = np.sin(sinu).astype(np.float32)
    COS = np.cos(sinu).astype(np.float32)
    qs = (LOG2_E / np.sqrt(np.float32(HD))
          * np.logaddexp(0.0, per_dim_scale.astype(np.float64))).astype(np.float32)
    qscale = (q_ln_scale * qs).astype(np.float32)
    kscale = k_ln_scale.astype(np.float32)

    # combined table [N, 256]: func f in {cosA,sinA,cosB,sinB} at cols
    # [64f:64f+64], q-scaled half at +0:32, k-scaled at +32:64
    blocks = []
    for base, half in ((COS, slice(0, HALF)), (SIN, slice(0, HALF)),
                       (COS, slice(HALF, HD)), (SIN, slice(HALF, HD))):
        blocks.append(base * qscale[None, half])
        blocks.append(base * kscale[None, half])
    return np.concatenate(blocks, axis=1).astype(np.float32)


def _host_masks():
    # mask[r, c] = 1 if c >= r  (S.T block: rows k, cols q)
    r = np.arange(128)[:, None]
    c = np.arange(128)[None, :]
    return (c >= r).astype(np.float32)


def kernel(**inputs):
    from concourse.bass_utils import run_bass_kernel_spmd

    nc = _get_nc()
    bf16 = _np_bf16()

    x = np.asarray(inputs["inputs_q"], dtype=np.float32)
    wq = np.asarray(inputs["wq"], dtype=np.float32)
    wk = np.asarray(inputs["wk"], dtype=np.float32)
    wv = np.asarray(inputs["wv"], dtype=np.float32)
    wo = np.asarray(inputs["wo"], dtype=np.float32)

    ctab = _host_tables(np.asarray(inputs["q_ln_scale"], np.float32),
                        np.asarray(inputs["k_ln_scale"], np.float32),
                        np.asarray(inputs["per_dim_scale"], np.float32))
    ctab = ctab.astype(bf16)
    masks = _host_masks().astype(bf16)

    in_maps = []
    for c in range(8):
        b, g = divmod(c, 4)
        hs = slice(NH * g, NH * (g + 1))
        wqkv_c = np.concatenate(
            [wq[:, hs, :].reshape(D, NH * HD),
             wk[:, hs, :].reshape(D, NH * HD),
             wv[:, hs, :].reshape(D, NH * HD)], axis=1)
        in_maps.append({
            "x": np.ascontiguousarray(x[b].T).astype(bf16),
            "wqkv": np.ascontiguousarray(wqkv_c).astype(bf16),
            "wo": np.ascontiguousarray(wo[hs].reshape(NH * HD, D)).astype(bf16),
            "ctab": ctab, "masks": masks,
            "ident": np.eye(128, dtype=bf16),
        })

    trace = os.environ.get("MHA_TRACE", "0") == "1"
    res = run_bass_kernel_spmd(nc, in_maps, list(range(8)), trace=trace)
    if trace:
        kernel.last_exec_time_ns = res.exec_time_ns
        kernel.last_results = res

    out = np.zeros((B, N, D), dtype=np.float32)
    for c in range(8):
        out[c // 4] += res.results[c]["outT"].T
    return out
